# revision 26
# baseline (speedup 1.0000x reference)
"""GCN encoder (3-layer, PyG GCNConv normalize=False + BN eval + ReLU) on 8 trn2 cores.

Strategy (node/dst-sharded, graph-parallel, flipped [feat, node] aggregation):
  - Nodes remapped into 8 cores x 49 tiles x 128 slots, balanced by in-degree.
    Table rows split into group A (tiles 0-23 of each core) and group B
    (tiles 24-48) so gather indices fit int16 and AllGathers can be chunked.
  - Layer 1 aggregates x directly (agg(x@W1.T) == agg(x)@W1.T): the full
    x table is staged per core, so no z1 compute phase and no first AllGather.
  - Aggregation per dst tile: edges sorted by local dst; each 128-edge chunk's
    scatter matrix is stored as a narrow [128, W] window (dst range of the
    sorted chunk) instead of a dense [128, 128] one-hot block -- ~6x less
    scatter-matrix HBM traffic. Chunk 0 is full-width with start=True (zeroes
    the PSUM bank); later chunks accumulate into column windows.
  - Flipped layout: matmul(lhsT=gathered z chunk [128e, 128f], rhs=mt window)
    accumulates aggT [128 feat, 128 nodes] per feature half. BN scale/bias are
    then per-partition, so the whole BN+ReLU epilogue is one scalar-engine
    activation per half, and z_next = h @ W.T needs no PE transposes.
  - AllGathers (z2, z3) are split into A/B chunks; chunk A is issued mid-layer
    to overlap with the remaining tiles' compute.
"""

import math
from dataclasses import dataclass

import ml_dtypes
import numpy as np

P = 128
HIDDEN = 256


@dataclass
class Cfg:
    n: int = 50000
    e: int = 1600000
    ncores: int = 8
    tiles: int = 49   # dst node tiles of 128 slots per core
    tiles_a: int = 24  # tiles in table group A (rest in group B)
    cin: int = 128

    @property
    def tiles_b(self) -> int:
        return self.tiles - self.tiles_a

    @property
    def slots_per_core(self) -> int:
        return self.tiles * P

    @property
    def total_slots(self) -> int:
        return self.ncores * self.slots_per_core

    @property
    def rows_a(self) -> int:
        return self.ncores * self.tiles_a * P

    @property
    def rows_b(self) -> int:
        return self.ncores * self.tiles_b * P


CFG = Cfg()


# ---------------------------------------------------------------------------
# Host-side preprocessing
# ---------------------------------------------------------------------------

def _balance_nodes(indeg: np.ndarray, cfg: Cfg) -> np.ndarray:
    """Assign each node a slot in [0, total_slots) so that each 128-slot tile
    has roughly equal total in-degree. Returns slot_of_node [n]."""
    import heapq

    nbins = cfg.ncores * cfg.tiles
    order = np.argsort(-indeg, kind="stable")
    heap = [(0, b) for b in range(nbins)]
    heapq.heapify(heap)
    counts = np.zeros(nbins, dtype=np.int64)
    slot_of = np.empty(cfg.n, dtype=np.int64)
    for v in order:
        load, b = heapq.heappop(heap)
        slot_of[v] = b * P + counts[b]
        counts[b] += 1
        load += int(indeg[v])
        if counts[b] < P:
            heapq.heappush(heap, (load, b))
    return slot_of


def _slot_to_table_row(s: np.ndarray, cfg: Cfg):
    """slot -> (group 0/1, row within that group's table)"""
    c = s // cfg.slots_per_core
    r = s % cfg.slots_per_core
    t = r // P
    lane = r % P
    grp = (t >= cfg.tiles_a).astype(np.int64)
    row_a = c * (cfg.tiles_a * P) + t * P + lane
    row_b = c * (cfg.tiles_b * P) + (t - cfg.tiles_a) * P + lane
    return grp, np.where(grp == 0, row_a, row_b)


def _prep(cfg: Cfg, x, edge_index, edge_attr, W1, b1, g1, beta1, m1, v1,
          W2, b2, g2, beta2, m2, v2, W3, b3):
    bf16 = ml_dtypes.bfloat16
    n, e = cfg.n, cfg.e
    T, TA = cfg.tiles, cfg.tiles_a
    src = np.asarray(edge_index[0], dtype=np.int64)
    dst = np.asarray(edge_index[1], dtype=np.int64)
    ew = np.asarray(edge_attr, dtype=np.float32).mean(axis=1)

    indeg = np.bincount(dst, minlength=n)
    slot_of = _balance_nodes(indeg, cfg)

    # Re-label which 24 tiles of each core form table-group A so that the
    # per-dst-tile A/B edge counts are balanced (reduces max chunk counts).
    # The swap only permutes tile indices within a core: kernel structure is
    # identical on all cores (SPMD); the mapping lives in host-side data.
    sbin0 = slot_of[src] // P
    dbin0 = slot_of[dst] // P
    nb = cfg.ncores * T
    cnt = np.zeros((nb, nb), dtype=np.int32)
    np.add.at(cnt, (sbin0, dbin0), 1)
    rng = np.random.default_rng(0)
    asel = np.zeros((cfg.ncores, T), dtype=bool)
    asel[:, :TA] = True
    a_cnt = cnt[asel.reshape(-1)].sum(axis=0).astype(np.int64)
    tot_cnt = cnt.sum(axis=0).astype(np.int64)

    def score(ac):
        bc = tot_cnt - ac
        return (int(np.ceil(ac.max() / P) + np.ceil(bc.max() / P)),
                int(ac.max() + bc.max()))

    best = score(a_cnt)
    for _ in range(4000):
        c = rng.integers(cfg.ncores)
        rows = np.flatnonzero(asel[c])
        rows_b = np.flatnonzero(~asel[c])
        i = rows[rng.integers(len(rows))]
        j = rows_b[rng.integers(len(rows_b))]
        gi, gj = c * T + i, c * T + j
        new_a = a_cnt - cnt[gi] + cnt[gj]
        s = score(new_a)
        if s <= best:
            best = s
            a_cnt = new_a
            asel[c, i] = False
            asel[c, j] = True
    # permute tile indices: A-set tiles -> 0..TA-1, rest -> TA..T-1
    perm = np.empty((cfg.ncores, T), dtype=np.int64)
    for c in range(cfg.ncores):
        a_tiles = np.flatnonzero(asel[c])
        b_tiles = np.flatnonzero(~asel[c])
        perm[c, a_tiles] = np.arange(TA)
        perm[c, b_tiles] = TA + np.arange(T - TA)
    s_core = slot_of // cfg.slots_per_core
    s_tile = (slot_of % cfg.slots_per_core) // P
    s_lane = slot_of % P
    slot_of = (s_core * cfg.slots_per_core
               + perm[s_core, s_tile] * P + s_lane)

    sslot = slot_of[src]
    dslot = slot_of[dst]
    sgrp, srow = _slot_to_table_row(sslot, cfg)
    ebin = dslot // P            # global tile id
    dlocal = dslot % P

    nbins = cfg.ncores * T
    key = ebin * 2 + sgrp
    order = np.lexsort((srow, dlocal, key))
    key_s = key[order]
    counts_g = np.bincount(key_s, minlength=nbins * 2)
    gstart = np.zeros(nbins * 2, dtype=np.int64)
    gstart[1:] = np.cumsum(counts_g)[:-1]
    rank = np.arange(e, dtype=np.int64) - gstart[key_s]

    ct_a = int(math.ceil(counts_g[0::2].max() / P))
    ct_b = int(math.ceil(counts_g[1::2].max() / P))
    ct = ct_a + ct_b

    e_bin = key_s // 2
    e_grp = key_s % 2
    e_chunk = rank // P + e_grp * ct_a
    e_srow = srow[order]
    e_dlocal = dlocal[order]
    e_w = ew[order].astype(np.float32)

    # re-sort lanes within each 128-edge chunk by src row (ascending gather
    # addresses per descriptor burst); chunk membership/windows unchanged
    cid = e_bin * ct + e_chunk
    order2 = np.lexsort((e_srow, cid))
    cid = cid[order2]
    e_bin = e_bin[order2]
    e_chunk = e_chunk[order2]
    e_srow = e_srow[order2]
    e_dlocal = e_dlocal[order2]
    e_w = e_w[order2]
    cstart = np.zeros(cid.max() + 2, dtype=np.int64)
    ccnt = np.bincount(cid, minlength=cid.max() + 1)
    cstart[1:] = np.cumsum(ccnt)
    e_lane = np.arange(e, dtype=np.int64) - cstart[cid]
    e_core = e_bin // T
    e_tile = e_bin % T

    # IDX[core, tile, chunk, lane] int16, pad = 0 (valid row, weight 0)
    idx = np.zeros((cfg.ncores, T, ct, P), dtype=np.int16)
    idx[e_core, e_tile, e_chunk, e_lane] = e_srow.astype(np.int16)

    # dst-window per (core, tile, chunk); offsets must be uniform across
    # cores (SPMD single program), so take min/max over cores.
    ncid = cfg.ncores * T * ct
    cid = e_bin * ct + e_chunk
    wmin = np.full(ncid, P, dtype=np.int64)
    wmax = np.full(ncid, -1, dtype=np.int64)
    np.minimum.at(wmin, cid, e_dlocal)
    np.maximum.at(wmax, cid, e_dlocal)
    wmin3 = wmin.reshape(cfg.ncores, T, ct)
    wmax3 = wmax.reshape(cfg.ncores, T, ct)
    lo = wmin3.min(axis=0)   # [T, ct]
    hi = wmax3.max(axis=0)
    mask = hi >= 0
    span = np.where(mask, hi - np.minimum(lo, P - 1) + 1, 1)
    W = int(span[:, 1:].max()) if ct > 1 else 1
    W = min(max(W, 8), P)
    woff = np.clip(np.where(mask, lo, 0), 0, P - W)   # [T, ct]
    woff[:, 0] = 0
    # coverage check: every edge's dlocal inside its chunk window
    full = (np.arange(ct) == 0)[None, :]
    wid = np.where(full, P, W)
    ok = (wmin3 >= woff[None]) & (wmax3 < woff[None] + wid[None])
    assert ok[wmax3 >= 0].all(), "window coverage failed"

    # scatter matrices, windowed: [core, tile, lane, CW]
    CW = P + (ct - 1) * W
    colbase = np.concatenate(([0], P + np.arange(ct - 1) * W))
    mtw = np.zeros((cfg.ncores, T, P, CW), dtype=np.float32)
    mt1 = np.zeros((cfg.ncores, T, P, CW), dtype=np.float32)
    e_col = colbase[e_chunk] + (e_dlocal - woff[e_tile, e_chunk])
    assert (e_col >= 0).all() and (e_col < CW).all()
    np.add.at(mtw, (e_core, e_tile, e_lane, e_col), e_w)
    np.add.at(mt1, (e_core, e_tile, e_lane, e_col), 1.0)
    mtw = mtw.astype(bf16)
    mt1 = mt1.astype(bf16)

    # gather-call index layout: group-major ([all tiles' A blocks | B blocks])
    # so pair-merged calls read contiguous columns; value at (partition p,
    # col s) = idx_linear[s*16 + p%16], replicated x8.
    idx_sb = np.zeros((cfg.ncores, P, T * ct * 8), dtype=np.int16)
    for g, ctg, off, base in ((0, ct_a, 0, 0), (1, ct_b, ct_a, T * ct_a * 8)):
        if ctg == 0:
            continue
        blk = idx[:, :, off:off + ctg, :].reshape(cfg.ncores, T, ctg * P)
        cols = blk.reshape(cfg.ncores, T, ctg * 8, 16)
        for tcol in range(ctg * 8):
            dst_col = base + np.arange(T) * (ctg * 8) + tcol
            idx_sb[:, :16, dst_col] = cols[:, :, tcol, :].transpose(0, 2, 1)
    idx_sb[:, 16:, :] = np.tile(idx_sb[:, :16, :], (1, 7, 1))

    # x table in A/B row order, bf16; pad rows -> 0 (same for all cores)
    sgrp_all, srow_all = _slot_to_table_row(np.arange(cfg.total_slots), cfg)
    trow_of_slot = np.where(sgrp_all == 0, srow_all, cfg.rows_a + srow_all)
    xfull = np.zeros((cfg.total_slots, cfg.cin), dtype=np.float32)
    xfull[trow_of_slot[slot_of]] = np.asarray(x, dtype=np.float32)
    xfull = np.ascontiguousarray(xfull.astype(bf16))

    node_of_slot = np.full(cfg.total_slots, -1, dtype=np.int64)
    node_of_slot[slot_of] = np.arange(n)

    # weights / epilogue params
    eps = 1e-5
    s1 = (np.asarray(g1) / np.sqrt(np.asarray(v1) + eps)).astype(np.float32)
    t1 = (np.asarray(beta1) + (np.asarray(b1) - np.asarray(m1)) * s1).astype(np.float32)
    s2 = (np.asarray(g2) / np.sqrt(np.asarray(v2) + eps)).astype(np.float32)
    t2 = (np.asarray(beta2) + (np.asarray(b2) - np.asarray(m2)) * s2).astype(np.float32)
    b3f = np.asarray(b3, np.float32)

    def halves(v):
        # [256] -> [P, 2] with [:, h] = v[h*128:(h+1)*128]
        return np.ascontiguousarray(np.asarray(v, np.float32).reshape(2, P).T)

    # w1t[h] = W1[h*128:(h+1)*128, :].T   [128 xf, 128 of]
    w1t = np.stack([np.asarray(W1, np.float32)[h * P:(h + 1) * P, :].T
                    for h in range(2)]).astype(bf16)
    # w2t[k] = W2.T[k*128:(k+1)*128, :]   [128 f_in, 256 out]
    w2t = np.asarray(W2, np.float32).T.reshape(2, P, HIDDEN).astype(bf16)
    w3t = np.asarray(W3, np.float32).T.reshape(2, P, HIDDEN).astype(bf16)

    in_maps = []
    for c in range(cfg.ncores):
        in_maps.append({
            "xfull": xfull,
            "idx": np.ascontiguousarray(idx_sb[c]),
            "mtw": np.ascontiguousarray(mtw[c]),
            "mt1": np.ascontiguousarray(mt1[c]),
            "w1t": w1t,
            "w2t": w2t,
            "w3t": w3t,
            "s1": halves(s1), "t1": halves(t1),
            "s2": halves(s2), "t2": halves(t2),
            "b3": halves(b3f),
        })
    return in_maps, node_of_slot, ct_a, ct_b, W, CW, woff


# ---------------------------------------------------------------------------
# Bass program
# ---------------------------------------------------------------------------

def _build(cfg: Cfg, ct_a: int, ct_b: int, W: int, CW: int, woff: np.ndarray):
    import concourse.mybir as mybir
    import concourse.tile as tile
    from concourse import bacc

    ct = ct_a + ct_b
    T, TA = cfg.tiles, cfg.tiles_a
    TB = cfg.tiles_b
    SPC = cfg.slots_per_core
    RA, RB = cfg.rows_a, cfg.rows_b
    DT = mybir.dt
    AF = mybir.ActivationFunctionType
    nc = bacc.Bacc("TRN2", target_bir_lowering=False, debug=False,
                   num_devices=cfg.ncores, num_swdge_queues=4)

    xfull_d = nc.declare_dram_parameter("xfull", [cfg.total_slots, cfg.cin], DT.bfloat16, isOutput=False)
    idx_d = nc.declare_dram_parameter("idx", [P, T * ct * 8], DT.int16, isOutput=False)
    mtw_d = nc.declare_dram_parameter("mtw", [T, P, CW], DT.bfloat16, isOutput=False)
    mt1_d = nc.declare_dram_parameter("mt1", [T, P, CW], DT.bfloat16, isOutput=False)
    w1t_d = nc.declare_dram_parameter("w1t", [2, P, P], DT.bfloat16, isOutput=False)
    w2t_d = nc.declare_dram_parameter("w2t", [2, P, HIDDEN], DT.bfloat16, isOutput=False)
    w3t_d = nc.declare_dram_parameter("w3t", [2, P, HIDDEN], DT.bfloat16, isOutput=False)
    bn_d = {}
    for nm in ("s1", "t1", "s2", "t2", "b3"):
        bn_d[nm] = nc.declare_dram_parameter(nm, [P, 2], DT.float32, isOutput=False)
    out_d = nc.declare_dram_parameter("out", [HIDDEN, SPC], DT.float32, isOutput=True)

    # z slices (this core's chunk-A / chunk-B rows) and gathered full tables
    zs = {}
    zf = {}
    for l in (2, 3):
        zs[(l, 0)] = nc.dram_tensor(f"zs{l}a", [TA * P, HIDDEN], DT.bfloat16)
        zs[(l, 1)] = nc.dram_tensor(f"zs{l}b", [TB * P, HIDDEN], DT.bfloat16)
        zf[(l, 0)] = nc.dram_tensor(f"zf{l}a", [RA, HIDDEN], DT.bfloat16,
                                    addr_space="Shared")
        zf[(l, 1)] = nc.dram_tensor(f"zf{l}b", [RB, HIDDEN], DT.bfloat16,
                                    addr_space="Shared")
    groups = [list(range(cfg.ncores))]

    def ag(l, g):
        nc.gpsimd.collective_compute(
            "AllGather", mybir.AluOpType.bypass, replica_groups=groups,
            ins=[zs[(l, g)][:]], outs=[zf[(l, g)][:]])

    with tile.TileContext(nc) as tc:
        with (
            tc.tile_pool(name="const", bufs=1) as const_pool,
            tc.tile_pool(name="mpool", bufs=6) as m_pool,
            tc.tile_pool(name="gpool", bufs=8) as g_pool,
            tc.tile_pool(name="hpool", bufs=3) as h_pool,
            tc.tile_pool(name="zpool", bufs=3) as z_pool,
            tc.tile_pool(name="opool", bufs=3) as o_pool,
            tc.tile_pool(name="apool", bufs=2) as a_pool,
            tc.tile_pool(name="agg_ps", bufs=3, space="PSUM") as agg_psum,
            tc.tile_pool(name="h_ps", bufs=2, space="PSUM") as h_psum,
            tc.tile_pool(name="z_ps", bufs=2, space="PSUM") as z_psum,
        ):
            # persistent tiles
            idx_sb = const_pool.tile([P, T * ct * 8], DT.int16)
            nc.sync.dma_start(idx_sb[:], idx_d[:])
            w1t_sb = const_pool.tile([P, 2, P], DT.bfloat16)
            nc.sync.dma_start(w1t_sb[:], w1t_d[:].rearrange("h p f -> p h f"))
            w2t_sb = const_pool.tile([P, 2, HIDDEN], DT.bfloat16)
            nc.sync.dma_start(w2t_sb[:], w2t_d[:].rearrange("h p n -> p h n"))
            w3t_sb = const_pool.tile([P, 2, HIDDEN], DT.bfloat16)
            nc.sync.dma_start(w3t_sb[:], w3t_d[:].rearrange("h p n -> p h n"))
            bn_sb = {}
            for nm in ("s1", "t1", "s2", "t2", "b3"):
                t_ = const_pool.tile([P, 2], DT.float32, tag=f"bn_{nm}")
                nc.sync.dma_start(t_[:], bn_d[nm][:])
                bn_sb[nm] = t_

            def agg_chain(t, ps, ga, ao, gb, bo, mt_sb, nfeat_half):
                """Accumulate aggT into ps: psH = ps[:, H*P:(H+1)*P]."""
                nh = nfeat_half
                for k in range(ct):
                    g = (ga[:, ao + k, :] if k < ct_a
                         else gb[:, bo + k - ct_a, :])
                    if k == 0:
                        co, wo, wd = 0, 0, P
                    else:
                        co = P + (k - 1) * W
                        wo, wd = int(woff[t, k]), W
                    for h in range(nh):
                        nc.tensor.matmul(
                            ps[:, h * P + wo: h * P + wo + wd],
                            g[:, h * P:(h + 1) * P] if nh > 1 else g,
                            mt_sb[:, co:co + wd],
                            start=(k == 0 and h == 0),
                            stop=(k == ct - 1 and h == nh - 1),
                            skip_group_check=True)

            def gather_grp(t0, nt, l, grp, elem, tag, bufs, q):
                """One gather covering `nt` consecutive tiles' group chunks."""
                if l == 1:
                    src = xfull_d[0:RA, :] if grp == 0 else xfull_d[RA:, :]
                else:
                    src = zf[(l, grp)][:]
                ctg = ct_a if grp == 0 else ct_b
                base = 0 if grp == 0 else T * ct_a * 8
                c0 = base + t0 * ctg * 8
                g = g_pool.tile([P, nt * ctg, elem], DT.bfloat16,
                                tag=f"g{'ab'[grp]}{tag}{nt}", bufs=bufs)
                nc.gpsimd.dma_gather(
                    g[:], src, idx_sb[:, c0: c0 + nt * ctg * 8],
                    nt * ctg * P, nt * ctg * P, elem, single_packet=False,
                    queue_num=q)
                return g

            def znext(t, hT, wnext_sb, zl):
                """z_{l+1} tile = h @ W.T -> DRAM slice (chunk A or B)."""
                zps = z_psum.tile([P, 512], DT.float32, tag="zps")
                for h in range(2):
                    nc.tensor.matmul(zps[:, 0:HIDDEN], hT[:, h, :],
                                     wnext_sb[:, h, :], start=(h == 0),
                                     stop=(h == 1), skip_group_check=True)
                zn = z_pool.tile([P, HIDDEN], DT.bfloat16, tag="zn")
                # copy + write both on the Activation engine so its queue is
                # self-contained and never blocks the sync queue's mt loads
                nc.scalar.copy(zn[:], zps[:, 0:HIDDEN])
                if t < TA:
                    nc.scalar.dma_start(zs[(zl, 0)][t * P:(t + 1) * P, :], zn[:])
                else:
                    nc.scalar.dma_start(zs[(zl, 1)][(t - TA) * P:(t - TA + 1) * P, :], zn[:])

            def l1_tile(t, ga, ao, gb, bo):
                mt_sb = m_pool.tile([P, CW], DT.bfloat16, tag="mt", bufs=4)
                nc.sync.dma_start(mt_sb[:], mtw_d[t])
                ps = agg_psum.tile([P, 512], DT.float32, tag="agg")
                agg_chain(t, ps, ga, ao, gb, bo, mt_sb, nfeat_half=1)
                aT = a_pool.tile([P, P], DT.bfloat16, tag="aT")
                nc.scalar.copy(aT[:], ps[:, 0:P])
                hps = h_psum.tile([P, 512], DT.float32, tag="hps")
                for h in range(2):
                    nc.tensor.matmul(hps[:, h * P:(h + 1) * P], w1t_sb[:, h, :],
                                     aT[:], start=(h == 0), stop=(h == 1),
                                     skip_group_check=True)
                hT = h_pool.tile([P, 2, P], DT.bfloat16, tag="hT")
                for h in range(2):
                    nc.scalar.activation(
                        hT[:, h, :], hps[:, h * P:(h + 1) * P], AF.Relu,
                        bias=bn_sb["t1"][:, h:h + 1], scale=bn_sb["s1"][:, h:h + 1])
                znext(t, hT, w2t_sb, 2)
                if t == TA + 12:
                    ag(2, 0)

            def l2_tile(t, ga, ao, gb, bo):
                mt_sb = m_pool.tile([P, CW], DT.bfloat16, tag="mt", bufs=4)
                nc.sync.dma_start(mt_sb[:], mtw_d[t])
                ps = agg_psum.tile([P, 512], DT.float32, tag="agg")
                agg_chain(t, ps, ga, ao, gb, bo, mt_sb, nfeat_half=2)
                hT = h_pool.tile([P, 2, P], DT.bfloat16, tag="hT")
                for h in range(2):
                    nc.scalar.activation(
                        hT[:, h, :], ps[:, h * P:(h + 1) * P], AF.Relu,
                        bias=bn_sb["t2"][:, h:h + 1], scale=bn_sb["s2"][:, h:h + 1])
                znext(t, hT, w3t_sb, 3)
                if t == TA + 12:
                    ag(3, 0)

            def l3_tile(t, ga, ao, gb, bo):
                mt_sb = m_pool.tile([P, CW], DT.bfloat16, tag="mt", bufs=4)
                nc.sync.dma_start(mt_sb[:], mt1_d[t])
                ps = agg_psum.tile([P, 512], DT.float32, tag="agg")
                agg_chain(t, ps, ga, ao, gb, bo, mt_sb, nfeat_half=2)
                ot = o_pool.tile([P, 2, P], DT.float32, tag="ot")
                for h in range(2):
                    nc.scalar.activation(
                        ot[:, h, :], ps[:, h * P:(h + 1) * P], AF.Identity,
                        bias=bn_sb["b3"][:, h:h + 1])
                for h in range(2):
                    nc.scalar.dma_start(
                        out_d[h * P:(h + 1) * P, t * P:(t + 1) * P], ot[:, h, :])

            # ---------------- layer 1 (aggregate x, then W1) ----------------
            for t in range(T):
                ga = gather_grp(t, 1, 1, 0, cfg.cin, "1", 4, (2 * t) % 4)
                gb = gather_grp(t, 1, 1, 1, cfg.cin, "1", 4, (2 * t + 1) % 4)
                l1_tile(t, ga, 0, gb, 0)
            ag(2, 1)

            # ---------------- layers 2, 3: pair-merged gathers ----------------
            for l, tile_fn in ((2, l2_tile), (3, l3_tile)):
                # pre-issue group-A pair gathers: they only need zf_a (AG-a,
                # done mid-previous-layer), so descgen overlaps the AG-b flight
                pend = {}
                for p in range(3):
                    pend[2 * p] = gather_grp(2 * p, 2, l, 0, HIDDEN, "23",
                                             3, (2 * p) % 4)
                for t0 in range(0, T, 2):
                    nt = min(2, T - t0)
                    p = t0 // 2
                    if t0 in pend:
                        ga = pend.pop(t0)
                    else:
                        ga = gather_grp(t0, nt, l, 0, HIDDEN, "23",
                                        3 if nt == 2 else 1, (2 * p) % 4)
                    gb = gather_grp(t0, nt, l, 1, HIDDEN, "23",
                                    3 if nt == 2 else 1, (2 * p + 1) % 4)
                    for i in range(nt):
                        tile_fn(t0 + i, ga, i * ct_a, gb, i * ct_b)
                if l == 2:
                    ag(3, 1)
    nc.compile()
    return nc


# ---------------------------------------------------------------------------
# Entry point
# ---------------------------------------------------------------------------

LAST_RESULTS = None  # BassKernelResults of the most recent _run (for profiling)


def _run(cfg: Cfg, inputs: dict, trace: bool = False,
         trace_cores=None) -> np.ndarray:
    global LAST_RESULTS
    from concourse.bass_utils import run_bass_kernel_spmd

    in_maps, node_of_slot, ct_a, ct_b, W, CW, woff = _prep(cfg, **inputs)
    nc = _build(cfg, ct_a, ct_b, W, CW, woff)
    kr = run_bass_kernel_spmd(nc, in_maps, list(range(cfg.ncores)), trace=trace,
                              trace_cores=trace_cores)
    LAST_RESULTS = kr
    res = kr.results
    # out per core: [HIDDEN, SPC] feature-major; assemble and transpose
    full = np.concatenate([res[c]["out"] for c in range(cfg.ncores)], axis=1)
    out = np.empty((cfg.n, HIDDEN), dtype=np.float32)
    valid = node_of_slot >= 0
    out[node_of_slot[valid]] = full[:, valid].T
    return out


def kernel(**inputs) -> np.ndarray:
    return _run(CFG, inputs)


# revision 27
# speedup vs baseline: 1.0177x; 1.0177x over previous
"""GCN encoder (3-layer, PyG GCNConv normalize=False + BN eval + ReLU) on 8 trn2 cores.

Strategy (node/dst-sharded, graph-parallel, flipped [feat, node] aggregation):
  - Nodes remapped into 8 cores x 49 tiles x 128 slots, balanced by in-degree.
    Table rows split into group A (tiles 0-23 of each core) and group B
    (tiles 24-48) so gather indices fit int16 and AllGathers can be chunked.
  - Layer 1 aggregates x directly (agg(x@W1.T) == agg(x)@W1.T): the full
    x table is staged per core, so no z1 compute phase and no first AllGather.
  - Aggregation per dst tile: edges sorted by local dst; each 128-edge chunk's
    scatter matrix is stored as a narrow [128, W] window (dst range of the
    sorted chunk) instead of a dense [128, 128] one-hot block -- ~6x less
    scatter-matrix HBM traffic. Chunk 0 is full-width with start=True (zeroes
    the PSUM bank); later chunks accumulate into column windows.
  - Flipped layout: matmul(lhsT=gathered z chunk [128e, 128f], rhs=mt window)
    accumulates aggT [128 feat, 128 nodes] per feature half. BN scale/bias are
    then per-partition, so the whole BN+ReLU epilogue is one scalar-engine
    activation per half, and z_next = h @ W.T needs no PE transposes.
  - AllGathers (z2, z3) are split into A/B chunks; chunk A is issued mid-layer
    to overlap with the remaining tiles' compute.
"""

import math
from dataclasses import dataclass

import ml_dtypes
import numpy as np

P = 128
HIDDEN = 256


@dataclass
class Cfg:
    n: int = 50000
    e: int = 1600000
    ncores: int = 8
    tiles: int = 49   # dst node tiles of 128 slots per core
    tiles_a: int = 24  # tiles in table group A (rest in group B)
    cin: int = 128

    @property
    def tiles_b(self) -> int:
        return self.tiles - self.tiles_a

    @property
    def slots_per_core(self) -> int:
        return self.tiles * P

    @property
    def total_slots(self) -> int:
        return self.ncores * self.slots_per_core

    @property
    def rows_a(self) -> int:
        return self.ncores * self.tiles_a * P

    @property
    def rows_b(self) -> int:
        return self.ncores * self.tiles_b * P


CFG = Cfg()


# ---------------------------------------------------------------------------
# Host-side preprocessing
# ---------------------------------------------------------------------------

def _balance_nodes(indeg: np.ndarray, cfg: Cfg) -> np.ndarray:
    """Assign each node a slot in [0, total_slots) so that each 128-slot tile
    has roughly equal total in-degree. Returns slot_of_node [n]."""
    import heapq

    nbins = cfg.ncores * cfg.tiles
    order = np.argsort(-indeg, kind="stable")
    heap = [(0, b) for b in range(nbins)]
    heapq.heapify(heap)
    counts = np.zeros(nbins, dtype=np.int64)
    slot_of = np.empty(cfg.n, dtype=np.int64)
    for v in order:
        load, b = heapq.heappop(heap)
        slot_of[v] = b * P + counts[b]
        counts[b] += 1
        load += int(indeg[v])
        if counts[b] < P:
            heapq.heappush(heap, (load, b))
    return slot_of


def _slot_to_table_row(s: np.ndarray, cfg: Cfg):
    """slot -> (group 0/1, row within that group's table)"""
    c = s // cfg.slots_per_core
    r = s % cfg.slots_per_core
    t = r // P
    lane = r % P
    grp = (t >= cfg.tiles_a).astype(np.int64)
    row_a = c * (cfg.tiles_a * P) + t * P + lane
    row_b = c * (cfg.tiles_b * P) + (t - cfg.tiles_a) * P + lane
    return grp, np.where(grp == 0, row_a, row_b)


def _prep(cfg: Cfg, x, edge_index, edge_attr, W1, b1, g1, beta1, m1, v1,
          W2, b2, g2, beta2, m2, v2, W3, b3):
    bf16 = ml_dtypes.bfloat16
    n, e = cfg.n, cfg.e
    T, TA = cfg.tiles, cfg.tiles_a
    src = np.asarray(edge_index[0], dtype=np.int64)
    dst = np.asarray(edge_index[1], dtype=np.int64)
    ew = np.asarray(edge_attr, dtype=np.float32).mean(axis=1)

    indeg = np.bincount(dst, minlength=n)
    slot_of = _balance_nodes(indeg, cfg)

    # Re-label which 24 tiles of each core form table-group A so that the
    # per-dst-tile A/B edge counts are balanced (reduces max chunk counts).
    # The swap only permutes tile indices within a core: kernel structure is
    # identical on all cores (SPMD); the mapping lives in host-side data.
    sbin0 = slot_of[src] // P
    dbin0 = slot_of[dst] // P
    nb = cfg.ncores * T
    cnt = np.zeros((nb, nb), dtype=np.int32)
    np.add.at(cnt, (sbin0, dbin0), 1)
    rng = np.random.default_rng(0)
    asel = np.zeros((cfg.ncores, T), dtype=bool)
    asel[:, :TA] = True
    a_cnt = cnt[asel.reshape(-1)].sum(axis=0).astype(np.int64)
    tot_cnt = cnt.sum(axis=0).astype(np.int64)

    def score(ac):
        bc = tot_cnt - ac
        return (int(np.ceil(ac.max() / P) + np.ceil(bc.max() / P)),
                int(ac.max() + bc.max()))

    best = score(a_cnt)
    for _ in range(4000):
        c = rng.integers(cfg.ncores)
        rows = np.flatnonzero(asel[c])
        rows_b = np.flatnonzero(~asel[c])
        i = rows[rng.integers(len(rows))]
        j = rows_b[rng.integers(len(rows_b))]
        gi, gj = c * T + i, c * T + j
        new_a = a_cnt - cnt[gi] + cnt[gj]
        s = score(new_a)
        if s <= best:
            best = s
            a_cnt = new_a
            asel[c, i] = False
            asel[c, j] = True
    # permute tile indices: A-set tiles -> 0..TA-1, rest -> TA..T-1
    perm = np.empty((cfg.ncores, T), dtype=np.int64)
    for c in range(cfg.ncores):
        a_tiles = np.flatnonzero(asel[c])
        b_tiles = np.flatnonzero(~asel[c])
        perm[c, a_tiles] = np.arange(TA)
        perm[c, b_tiles] = TA + np.arange(T - TA)
    s_core = slot_of // cfg.slots_per_core
    s_tile = (slot_of % cfg.slots_per_core) // P
    s_lane = slot_of % P
    slot_of = (s_core * cfg.slots_per_core
               + perm[s_core, s_tile] * P + s_lane)

    sslot = slot_of[src]
    dslot = slot_of[dst]
    sgrp, srow = _slot_to_table_row(sslot, cfg)
    ebin = dslot // P            # global tile id
    dlocal = dslot % P

    nbins = cfg.ncores * T
    key = ebin * 2 + sgrp
    order = np.lexsort((srow, dlocal, key))
    key_s = key[order]
    counts_g = np.bincount(key_s, minlength=nbins * 2)
    gstart = np.zeros(nbins * 2, dtype=np.int64)
    gstart[1:] = np.cumsum(counts_g)[:-1]
    rank = np.arange(e, dtype=np.int64) - gstart[key_s]

    ct_a = int(math.ceil(counts_g[0::2].max() / P))
    ct_b = int(math.ceil(counts_g[1::2].max() / P))
    ct = ct_a + ct_b

    e_bin = key_s // 2
    e_grp = key_s % 2
    e_chunk = rank // P + e_grp * ct_a
    e_srow = srow[order]
    e_dlocal = dlocal[order]
    e_w = ew[order].astype(np.float32)

    # re-sort lanes within each 128-edge chunk by src row (ascending gather
    # addresses per descriptor burst); chunk membership/windows unchanged
    cid = e_bin * ct + e_chunk
    order2 = np.lexsort((e_srow, cid))
    cid = cid[order2]
    e_bin = e_bin[order2]
    e_chunk = e_chunk[order2]
    e_srow = e_srow[order2]
    e_dlocal = e_dlocal[order2]
    e_w = e_w[order2]
    cstart = np.zeros(cid.max() + 2, dtype=np.int64)
    ccnt = np.bincount(cid, minlength=cid.max() + 1)
    cstart[1:] = np.cumsum(ccnt)
    e_lane = np.arange(e, dtype=np.int64) - cstart[cid]
    e_core = e_bin // T
    e_tile = e_bin % T

    # IDX[core, tile, chunk, lane] int16, pad = 0 (valid row, weight 0)
    idx = np.zeros((cfg.ncores, T, ct, P), dtype=np.int16)
    idx[e_core, e_tile, e_chunk, e_lane] = e_srow.astype(np.int16)

    # dst-window per (core, tile, chunk); offsets must be uniform across
    # cores (SPMD single program), so take min/max over cores.
    ncid = cfg.ncores * T * ct
    cid = e_bin * ct + e_chunk
    wmin = np.full(ncid, P, dtype=np.int64)
    wmax = np.full(ncid, -1, dtype=np.int64)
    np.minimum.at(wmin, cid, e_dlocal)
    np.maximum.at(wmax, cid, e_dlocal)
    wmin3 = wmin.reshape(cfg.ncores, T, ct)
    wmax3 = wmax.reshape(cfg.ncores, T, ct)
    lo = wmin3.min(axis=0)   # [T, ct]
    hi = wmax3.max(axis=0)
    mask = hi >= 0
    span = np.where(mask, hi - np.minimum(lo, P - 1) + 1, 1)
    W = int(span[:, 1:].max()) if ct > 1 else 1
    W = min(max(W, 8), P)
    woff = np.clip(np.where(mask, lo, 0), 0, P - W)   # [T, ct]
    woff[:, 0] = 0
    # coverage check: every edge's dlocal inside its chunk window
    full = (np.arange(ct) == 0)[None, :]
    wid = np.where(full, P, W)
    ok = (wmin3 >= woff[None]) & (wmax3 < woff[None] + wid[None])
    assert ok[wmax3 >= 0].all(), "window coverage failed"

    # scatter matrices, windowed: [core, tile, lane, CW]
    CW = P + (ct - 1) * W
    colbase = np.concatenate(([0], P + np.arange(ct - 1) * W))
    mtw = np.zeros((cfg.ncores, T, P, CW), dtype=np.float32)
    mt1 = np.zeros((cfg.ncores, T, P, CW), dtype=np.float32)
    e_col = colbase[e_chunk] + (e_dlocal - woff[e_tile, e_chunk])
    assert (e_col >= 0).all() and (e_col < CW).all()
    np.add.at(mtw, (e_core, e_tile, e_lane, e_col), e_w)
    np.add.at(mt1, (e_core, e_tile, e_lane, e_col), 1.0)
    mtw = mtw.astype(bf16)
    mt1 = mt1.astype(bf16)

    # gather-call index layout: group-major ([all tiles' A blocks | B blocks])
    # so pair-merged calls read contiguous columns; value at (partition p,
    # col s) = idx_linear[s*16 + p%16], replicated x8.
    idx_sb = np.zeros((cfg.ncores, P, T * ct * 8), dtype=np.int16)
    for g, ctg, off, base in ((0, ct_a, 0, 0), (1, ct_b, ct_a, T * ct_a * 8)):
        if ctg == 0:
            continue
        blk = idx[:, :, off:off + ctg, :].reshape(cfg.ncores, T, ctg * P)
        cols = blk.reshape(cfg.ncores, T, ctg * 8, 16)
        for tcol in range(ctg * 8):
            dst_col = base + np.arange(T) * (ctg * 8) + tcol
            idx_sb[:, :16, dst_col] = cols[:, :, tcol, :].transpose(0, 2, 1)
    idx_sb[:, 16:, :] = np.tile(idx_sb[:, :16, :], (1, 7, 1))

    # x table in A/B row order, bf16; pad rows -> 0 (same for all cores)
    sgrp_all, srow_all = _slot_to_table_row(np.arange(cfg.total_slots), cfg)
    trow_of_slot = np.where(sgrp_all == 0, srow_all, cfg.rows_a + srow_all)
    xfull = np.zeros((cfg.total_slots, cfg.cin), dtype=np.float32)
    xfull[trow_of_slot[slot_of]] = np.asarray(x, dtype=np.float32)
    xfull = np.ascontiguousarray(xfull.astype(bf16))

    node_of_slot = np.full(cfg.total_slots, -1, dtype=np.int64)
    node_of_slot[slot_of] = np.arange(n)

    # weights / epilogue params
    eps = 1e-5
    s1 = (np.asarray(g1) / np.sqrt(np.asarray(v1) + eps)).astype(np.float32)
    t1 = (np.asarray(beta1) + (np.asarray(b1) - np.asarray(m1)) * s1).astype(np.float32)
    s2 = (np.asarray(g2) / np.sqrt(np.asarray(v2) + eps)).astype(np.float32)
    t2 = (np.asarray(beta2) + (np.asarray(b2) - np.asarray(m2)) * s2).astype(np.float32)
    b3f = np.asarray(b3, np.float32)

    def halves(v):
        # [256] -> [P, 2] with [:, h] = v[h*128:(h+1)*128]
        return np.ascontiguousarray(np.asarray(v, np.float32).reshape(2, P).T)

    # w1t[h] = W1[h*128:(h+1)*128, :].T   [128 xf, 128 of]
    w1t = np.stack([np.asarray(W1, np.float32)[h * P:(h + 1) * P, :].T
                    for h in range(2)]).astype(bf16)
    # w2t[k] = W2.T[k*128:(k+1)*128, :]   [128 f_in, 256 out]
    w2t = np.asarray(W2, np.float32).T.reshape(2, P, HIDDEN).astype(bf16)
    w3t = np.asarray(W3, np.float32).T.reshape(2, P, HIDDEN).astype(bf16)

    in_maps = []
    for c in range(cfg.ncores):
        in_maps.append({
            "xfull": xfull,
            "idx": np.ascontiguousarray(idx_sb[c]),
            "mtw": np.ascontiguousarray(mtw[c]),
            "mt1": np.ascontiguousarray(mt1[c]),
            "w1t": w1t,
            "w2t": w2t,
            "w3t": w3t,
            "s1": halves(s1), "t1": halves(t1),
            "s2": halves(s2), "t2": halves(t2),
            "b3": halves(b3f),
        })
    return in_maps, node_of_slot, ct_a, ct_b, W, CW, woff


# ---------------------------------------------------------------------------
# Bass program
# ---------------------------------------------------------------------------

def _build(cfg: Cfg, ct_a: int, ct_b: int, W: int, CW: int, woff: np.ndarray):
    import concourse.mybir as mybir
    import concourse.tile as tile
    from concourse import bacc

    ct = ct_a + ct_b
    T, TA = cfg.tiles, cfg.tiles_a
    TB = cfg.tiles_b
    SPC = cfg.slots_per_core
    RA, RB = cfg.rows_a, cfg.rows_b
    DT = mybir.dt
    AF = mybir.ActivationFunctionType
    nc = bacc.Bacc("TRN2", target_bir_lowering=False, debug=False,
                   num_devices=cfg.ncores, num_swdge_queues=4,
                   dynamic_dma_scratch_size=24576)

    xfull_d = nc.declare_dram_parameter("xfull", [cfg.total_slots, cfg.cin], DT.bfloat16, isOutput=False)
    idx_d = nc.declare_dram_parameter("idx", [P, T * ct * 8], DT.int16, isOutput=False)
    mtw_d = nc.declare_dram_parameter("mtw", [T, P, CW], DT.bfloat16, isOutput=False)
    mt1_d = nc.declare_dram_parameter("mt1", [T, P, CW], DT.bfloat16, isOutput=False)
    w1t_d = nc.declare_dram_parameter("w1t", [2, P, P], DT.bfloat16, isOutput=False)
    w2t_d = nc.declare_dram_parameter("w2t", [2, P, HIDDEN], DT.bfloat16, isOutput=False)
    w3t_d = nc.declare_dram_parameter("w3t", [2, P, HIDDEN], DT.bfloat16, isOutput=False)
    bn_d = {}
    for nm in ("s1", "t1", "s2", "t2", "b3"):
        bn_d[nm] = nc.declare_dram_parameter(nm, [P, 2], DT.float32, isOutput=False)
    out_d = nc.declare_dram_parameter("out", [HIDDEN, SPC], DT.float32, isOutput=True)

    # z slices (this core's chunk-A / chunk-B rows) and gathered full tables
    zs = {}
    zf = {}
    for l in (2, 3):
        zs[(l, 0)] = nc.dram_tensor(f"zs{l}a", [TA * P, HIDDEN], DT.bfloat16)
        zs[(l, 1)] = nc.dram_tensor(f"zs{l}b", [TB * P, HIDDEN], DT.bfloat16)
        zf[(l, 0)] = nc.dram_tensor(f"zf{l}a", [RA, HIDDEN], DT.bfloat16,
                                    addr_space="Shared")
        zf[(l, 1)] = nc.dram_tensor(f"zf{l}b", [RB, HIDDEN], DT.bfloat16,
                                    addr_space="Shared")
    groups = [list(range(cfg.ncores))]

    def ag(l, g):
        nc.gpsimd.collective_compute(
            "AllGather", mybir.AluOpType.bypass, replica_groups=groups,
            ins=[zs[(l, g)][:]], outs=[zf[(l, g)][:]])

    with tile.TileContext(nc) as tc:
        with (
            tc.tile_pool(name="const", bufs=1) as const_pool,
            tc.tile_pool(name="mpool", bufs=6) as m_pool,
            tc.tile_pool(name="gpool", bufs=8) as g_pool,
            tc.tile_pool(name="hpool", bufs=3) as h_pool,
            tc.tile_pool(name="zpool", bufs=3) as z_pool,
            tc.tile_pool(name="opool", bufs=3) as o_pool,
            tc.tile_pool(name="apool", bufs=2) as a_pool,
            tc.tile_pool(name="agg_ps", bufs=3, space="PSUM") as agg_psum,
            tc.tile_pool(name="h_ps", bufs=2, space="PSUM") as h_psum,
            tc.tile_pool(name="z_ps", bufs=2, space="PSUM") as z_psum,
        ):
            # persistent tiles
            idx_sb = const_pool.tile([P, T * ct * 8], DT.int16)
            nc.sync.dma_start(idx_sb[:], idx_d[:])
            w1t_sb = const_pool.tile([P, 2, P], DT.bfloat16)
            nc.sync.dma_start(w1t_sb[:], w1t_d[:].rearrange("h p f -> p h f"))
            w2t_sb = const_pool.tile([P, 2, HIDDEN], DT.bfloat16)
            nc.sync.dma_start(w2t_sb[:], w2t_d[:].rearrange("h p n -> p h n"))
            w3t_sb = const_pool.tile([P, 2, HIDDEN], DT.bfloat16)
            nc.sync.dma_start(w3t_sb[:], w3t_d[:].rearrange("h p n -> p h n"))
            bn_sb = {}
            for nm in ("s1", "t1", "s2", "t2", "b3"):
                t_ = const_pool.tile([P, 2], DT.float32, tag=f"bn_{nm}")
                nc.sync.dma_start(t_[:], bn_d[nm][:])
                bn_sb[nm] = t_

            def agg_chain(t, ps, ga, ao, gb, bo, mt_sb, nfeat_half):
                """Accumulate aggT into ps: psH = ps[:, H*P:(H+1)*P]."""
                nh = nfeat_half
                for k in range(ct):
                    g = (ga[:, ao + k, :] if k < ct_a
                         else gb[:, bo + k - ct_a, :])
                    if k == 0:
                        co, wo, wd = 0, 0, P
                    else:
                        co = P + (k - 1) * W
                        wo, wd = int(woff[t, k]), W
                    for h in range(nh):
                        nc.tensor.matmul(
                            ps[:, h * P + wo: h * P + wo + wd],
                            g[:, h * P:(h + 1) * P] if nh > 1 else g,
                            mt_sb[:, co:co + wd],
                            start=(k == 0 and h == 0),
                            stop=(k == ct - 1 and h == nh - 1),
                            skip_group_check=True)

            def gather_grp(t0, nt, l, grp, elem, tag, bufs, q):
                """One gather covering `nt` consecutive tiles' group chunks."""
                if l == 1:
                    src = xfull_d[0:RA, :] if grp == 0 else xfull_d[RA:, :]
                else:
                    src = zf[(l, grp)][:]
                ctg = ct_a if grp == 0 else ct_b
                base = 0 if grp == 0 else T * ct_a * 8
                c0 = base + t0 * ctg * 8
                g = g_pool.tile([P, nt * ctg, elem], DT.bfloat16,
                                tag=f"g{'ab'[grp]}{tag}{nt}", bufs=bufs)
                nc.gpsimd.dma_gather(
                    g[:], src, idx_sb[:, c0: c0 + nt * ctg * 8],
                    nt * ctg * P, nt * ctg * P, elem, single_packet=False,
                    queue_num=q)
                return g

            def znext(t, hT, wnext_sb, zl):
                """z_{l+1} tile = h @ W.T -> DRAM slice (chunk A or B)."""
                zps = z_psum.tile([P, 512], DT.float32, tag="zps")
                for h in range(2):
                    nc.tensor.matmul(zps[:, 0:HIDDEN], hT[:, h, :],
                                     wnext_sb[:, h, :], start=(h == 0),
                                     stop=(h == 1), skip_group_check=True)
                zn = z_pool.tile([P, HIDDEN], DT.bfloat16, tag="zn")
                # copy + write both on the Activation engine so its queue is
                # self-contained and never blocks the sync queue's mt loads
                nc.scalar.copy(zn[:], zps[:, 0:HIDDEN])
                if t < TA:
                    nc.scalar.dma_start(zs[(zl, 0)][t * P:(t + 1) * P, :], zn[:])
                else:
                    nc.scalar.dma_start(zs[(zl, 1)][(t - TA) * P:(t - TA + 1) * P, :], zn[:])

            def l1_tile(t, ga, ao, gb, bo):
                mt_sb = m_pool.tile([P, CW], DT.bfloat16, tag="mt", bufs=4)
                nc.sync.dma_start(mt_sb[:], mtw_d[t])
                ps = agg_psum.tile([P, 512], DT.float32, tag="agg")
                agg_chain(t, ps, ga, ao, gb, bo, mt_sb, nfeat_half=1)
                aT = a_pool.tile([P, P], DT.bfloat16, tag="aT")
                nc.scalar.copy(aT[:], ps[:, 0:P])
                hps = h_psum.tile([P, 512], DT.float32, tag="hps")
                for h in range(2):
                    nc.tensor.matmul(hps[:, h * P:(h + 1) * P], w1t_sb[:, h, :],
                                     aT[:], start=(h == 0), stop=(h == 1),
                                     skip_group_check=True)
                hT = h_pool.tile([P, 2, P], DT.bfloat16, tag="hT")
                for h in range(2):
                    nc.scalar.activation(
                        hT[:, h, :], hps[:, h * P:(h + 1) * P], AF.Relu,
                        bias=bn_sb["t1"][:, h:h + 1], scale=bn_sb["s1"][:, h:h + 1])
                znext(t, hT, w2t_sb, 2)
                if t == TA + 12:
                    ag(2, 0)

            def l2_tile(t, ga, ao, gb, bo):
                mt_sb = m_pool.tile([P, CW], DT.bfloat16, tag="mt", bufs=4)
                nc.sync.dma_start(mt_sb[:], mtw_d[t])
                ps = agg_psum.tile([P, 512], DT.float32, tag="agg")
                agg_chain(t, ps, ga, ao, gb, bo, mt_sb, nfeat_half=2)
                hT = h_pool.tile([P, 2, P], DT.bfloat16, tag="hT")
                for h in range(2):
                    nc.scalar.activation(
                        hT[:, h, :], ps[:, h * P:(h + 1) * P], AF.Relu,
                        bias=bn_sb["t2"][:, h:h + 1], scale=bn_sb["s2"][:, h:h + 1])
                znext(t, hT, w3t_sb, 3)
                if t == TA + 12:
                    ag(3, 0)

            def l3_tile(t, ga, ao, gb, bo):
                mt_sb = m_pool.tile([P, CW], DT.bfloat16, tag="mt", bufs=4)
                nc.sync.dma_start(mt_sb[:], mt1_d[t])
                ps = agg_psum.tile([P, 512], DT.float32, tag="agg")
                agg_chain(t, ps, ga, ao, gb, bo, mt_sb, nfeat_half=2)
                ot = o_pool.tile([P, 2, P], DT.float32, tag="ot")
                for h in range(2):
                    nc.scalar.activation(
                        ot[:, h, :], ps[:, h * P:(h + 1) * P], AF.Identity,
                        bias=bn_sb["b3"][:, h:h + 1])
                for h in range(2):
                    nc.scalar.dma_start(
                        out_d[h * P:(h + 1) * P, t * P:(t + 1) * P], ot[:, h, :])

            # ---------------- layer 1 (aggregate x, then W1) ----------------
            for t in range(T):
                ga = gather_grp(t, 1, 1, 0, cfg.cin, "1", 4, (2 * t) % 4)
                gb = gather_grp(t, 1, 1, 1, cfg.cin, "1", 4, (2 * t + 1) % 4)
                l1_tile(t, ga, 0, gb, 0)
            ag(2, 1)

            # ---------------- layers 2, 3: pair-merged gathers ----------------
            for l, tile_fn in ((2, l2_tile), (3, l3_tile)):
                # pre-issue group-A pair gathers: they only need zf_a (AG-a,
                # done mid-previous-layer), so descgen overlaps the AG-b flight
                pend = {}
                for p in range(3):
                    pend[2 * p] = gather_grp(2 * p, 2, l, 0, HIDDEN, "23",
                                             3, (2 * p) % 4)
                for t0 in range(0, T, 2):
                    nt = min(2, T - t0)
                    p = t0 // 2
                    if t0 in pend:
                        ga = pend.pop(t0)
                    else:
                        ga = gather_grp(t0, nt, l, 0, HIDDEN, "23",
                                        3 if nt == 2 else 1, (2 * p) % 4)
                    gb = gather_grp(t0, nt, l, 1, HIDDEN, "23",
                                    3 if nt == 2 else 1, (2 * p + 1) % 4)
                    for i in range(nt):
                        tile_fn(t0 + i, ga, i * ct_a, gb, i * ct_b)
                if l == 2:
                    ag(3, 1)
    nc.compile()
    return nc


# ---------------------------------------------------------------------------
# Entry point
# ---------------------------------------------------------------------------

LAST_RESULTS = None  # BassKernelResults of the most recent _run (for profiling)


def _run(cfg: Cfg, inputs: dict, trace: bool = False,
         trace_cores=None) -> np.ndarray:
    global LAST_RESULTS
    from concourse.bass_utils import run_bass_kernel_spmd

    in_maps, node_of_slot, ct_a, ct_b, W, CW, woff = _prep(cfg, **inputs)
    nc = _build(cfg, ct_a, ct_b, W, CW, woff)
    kr = run_bass_kernel_spmd(nc, in_maps, list(range(cfg.ncores)), trace=trace,
                              trace_cores=trace_cores)
    LAST_RESULTS = kr
    res = kr.results
    # out per core: [HIDDEN, SPC] feature-major; assemble and transpose
    full = np.concatenate([res[c]["out"] for c in range(cfg.ncores)], axis=1)
    out = np.empty((cfg.n, HIDDEN), dtype=np.float32)
    valid = node_of_slot >= 0
    out[node_of_slot[valid]] = full[:, valid].T
    return out


def kernel(**inputs) -> np.ndarray:
    return _run(CFG, inputs)


# revision 28
# speedup vs baseline: 1.0531x; 1.0347x over previous
"""GCN encoder (3-layer, PyG GCNConv normalize=False + BN eval + ReLU) on 8 trn2 cores.

Strategy (node/dst-sharded, graph-parallel, flipped [feat, node] aggregation):
  - Nodes remapped into 8 cores x 49 tiles x 128 slots, balanced by in-degree.
    Table rows split into group A (tiles 0-23 of each core) and group B
    (tiles 24-48) so gather indices fit int16 and AllGathers can be chunked.
  - Layer 1 aggregates x directly (agg(x@W1.T) == agg(x)@W1.T): the full
    x table is staged per core, so no z1 compute phase and no first AllGather.
  - Aggregation per dst tile: edges sorted by local dst; each 128-edge chunk's
    scatter matrix is stored as a narrow [128, W] window (dst range of the
    sorted chunk) instead of a dense [128, 128] one-hot block -- ~6x less
    scatter-matrix HBM traffic. Chunk 0 is full-width with start=True (zeroes
    the PSUM bank); later chunks accumulate into column windows.
  - Flipped layout: matmul(lhsT=gathered z chunk [128e, 128f], rhs=mt window)
    accumulates aggT [128 feat, 128 nodes] per feature half. BN scale/bias are
    then per-partition, so the whole BN+ReLU epilogue is one scalar-engine
    activation per half, and z_next = h @ W.T needs no PE transposes.
  - AllGathers (z2, z3) are split into A/B chunks; chunk A is issued mid-layer
    to overlap with the remaining tiles' compute.
"""

import math
from dataclasses import dataclass

import ml_dtypes
import numpy as np

P = 128
HIDDEN = 256


@dataclass
class Cfg:
    n: int = 50000
    e: int = 1600000
    ncores: int = 8
    tiles: int = 49   # dst node tiles of 128 slots per core
    tiles_a: int = 24  # tiles in table group A (rest in group B)
    cin: int = 128

    @property
    def tiles_b(self) -> int:
        return self.tiles - self.tiles_a

    @property
    def slots_per_core(self) -> int:
        return self.tiles * P

    @property
    def total_slots(self) -> int:
        return self.ncores * self.slots_per_core

    @property
    def rows_a(self) -> int:
        return self.ncores * self.tiles_a * P

    @property
    def rows_b(self) -> int:
        return self.ncores * self.tiles_b * P


CFG = Cfg()


# ---------------------------------------------------------------------------
# Host-side preprocessing
# ---------------------------------------------------------------------------

def _balance_nodes(indeg: np.ndarray, cfg: Cfg) -> np.ndarray:
    """Assign each node a slot in [0, total_slots) so that each 128-slot tile
    has roughly equal total in-degree. Returns slot_of_node [n]."""
    import heapq

    nbins = cfg.ncores * cfg.tiles
    order = np.argsort(-indeg, kind="stable")
    heap = [(0, b) for b in range(nbins)]
    heapq.heapify(heap)
    counts = np.zeros(nbins, dtype=np.int64)
    slot_of = np.empty(cfg.n, dtype=np.int64)
    for v in order:
        load, b = heapq.heappop(heap)
        slot_of[v] = b * P + counts[b]
        counts[b] += 1
        load += int(indeg[v])
        if counts[b] < P:
            heapq.heappush(heap, (load, b))
    return slot_of


def _slot_to_table_row(s: np.ndarray, cfg: Cfg):
    """slot -> (group 0/1, row within that group's table)"""
    c = s // cfg.slots_per_core
    r = s % cfg.slots_per_core
    t = r // P
    lane = r % P
    grp = (t >= cfg.tiles_a).astype(np.int64)
    row_a = c * (cfg.tiles_a * P) + t * P + lane
    row_b = c * (cfg.tiles_b * P) + (t - cfg.tiles_a) * P + lane
    return grp, np.where(grp == 0, row_a, row_b)


def _prep(cfg: Cfg, x, edge_index, edge_attr, W1, b1, g1, beta1, m1, v1,
          W2, b2, g2, beta2, m2, v2, W3, b3):
    bf16 = ml_dtypes.bfloat16
    n, e = cfg.n, cfg.e
    T, TA = cfg.tiles, cfg.tiles_a
    src = np.asarray(edge_index[0], dtype=np.int64)
    dst = np.asarray(edge_index[1], dtype=np.int64)
    ew = np.asarray(edge_attr, dtype=np.float32).mean(axis=1)

    indeg = np.bincount(dst, minlength=n)
    slot_of = _balance_nodes(indeg, cfg)

    # Re-label which 24 tiles of each core form table-group A so that the
    # per-dst-tile A/B edge counts are balanced (reduces max chunk counts).
    # The swap only permutes tile indices within a core: kernel structure is
    # identical on all cores (SPMD); the mapping lives in host-side data.
    sbin0 = slot_of[src] // P
    dbin0 = slot_of[dst] // P
    nb = cfg.ncores * T
    cnt = np.zeros((nb, nb), dtype=np.int32)
    np.add.at(cnt, (sbin0, dbin0), 1)
    rng = np.random.default_rng(0)
    asel = np.zeros((cfg.ncores, T), dtype=bool)
    asel[:, :TA] = True
    a_cnt = cnt[asel.reshape(-1)].sum(axis=0).astype(np.int64)
    tot_cnt = cnt.sum(axis=0).astype(np.int64)

    def score(ac):
        bc = tot_cnt - ac
        return (int(np.ceil(ac.max() / P) + np.ceil(bc.max() / P)),
                int(ac.max() + bc.max()))

    best = score(a_cnt)
    for _ in range(4000):
        c = rng.integers(cfg.ncores)
        rows = np.flatnonzero(asel[c])
        rows_b = np.flatnonzero(~asel[c])
        i = rows[rng.integers(len(rows))]
        j = rows_b[rng.integers(len(rows_b))]
        gi, gj = c * T + i, c * T + j
        new_a = a_cnt - cnt[gi] + cnt[gj]
        s = score(new_a)
        if s <= best:
            best = s
            a_cnt = new_a
            asel[c, i] = False
            asel[c, j] = True
    # permute tile indices: A-set tiles -> 0..TA-1, rest -> TA..T-1
    perm = np.empty((cfg.ncores, T), dtype=np.int64)
    for c in range(cfg.ncores):
        a_tiles = np.flatnonzero(asel[c])
        b_tiles = np.flatnonzero(~asel[c])
        perm[c, a_tiles] = np.arange(TA)
        perm[c, b_tiles] = TA + np.arange(T - TA)
    s_core = slot_of // cfg.slots_per_core
    s_tile = (slot_of % cfg.slots_per_core) // P
    s_lane = slot_of % P
    slot_of = (s_core * cfg.slots_per_core
               + perm[s_core, s_tile] * P + s_lane)

    sslot = slot_of[src]
    dslot = slot_of[dst]
    sgrp, srow = _slot_to_table_row(sslot, cfg)
    ebin = dslot // P            # global tile id
    dlocal = dslot % P

    nbins = cfg.ncores * T
    key = ebin * 2 + sgrp
    order = np.lexsort((srow, dlocal, key))
    key_s = key[order]
    counts_g = np.bincount(key_s, minlength=nbins * 2)
    gstart = np.zeros(nbins * 2, dtype=np.int64)
    gstart[1:] = np.cumsum(counts_g)[:-1]
    rank = np.arange(e, dtype=np.int64) - gstart[key_s]

    ct_a = int(math.ceil(counts_g[0::2].max() / P))
    ct_b = int(math.ceil(counts_g[1::2].max() / P))
    ct = ct_a + ct_b

    e_bin = key_s // 2
    e_grp = key_s % 2
    e_chunk = rank // P + e_grp * ct_a
    e_srow = srow[order]
    e_dlocal = dlocal[order]
    e_w = ew[order].astype(np.float32)

    # re-sort lanes within each 128-edge chunk by src row (ascending gather
    # addresses per descriptor burst); chunk membership/windows unchanged
    cid = e_bin * ct + e_chunk
    order2 = np.lexsort((e_srow, cid))
    cid = cid[order2]
    e_bin = e_bin[order2]
    e_chunk = e_chunk[order2]
    e_srow = e_srow[order2]
    e_dlocal = e_dlocal[order2]
    e_w = e_w[order2]
    cstart = np.zeros(cid.max() + 2, dtype=np.int64)
    ccnt = np.bincount(cid, minlength=cid.max() + 1)
    cstart[1:] = np.cumsum(ccnt)
    e_lane = np.arange(e, dtype=np.int64) - cstart[cid]
    e_core = e_bin // T
    e_tile = e_bin % T

    # IDX[core, tile, chunk, lane] int16, pad = 0 (valid row, weight 0)
    idx = np.zeros((cfg.ncores, T, ct, P), dtype=np.int16)
    idx[e_core, e_tile, e_chunk, e_lane] = e_srow.astype(np.int16)

    # dst-window per (core, tile, chunk); offsets must be uniform across
    # cores (SPMD single program), so take min/max over cores.
    ncid = cfg.ncores * T * ct
    cid = e_bin * ct + e_chunk
    wmin = np.full(ncid, P, dtype=np.int64)
    wmax = np.full(ncid, -1, dtype=np.int64)
    np.minimum.at(wmin, cid, e_dlocal)
    np.maximum.at(wmax, cid, e_dlocal)
    wmin3 = wmin.reshape(cfg.ncores, T, ct)
    wmax3 = wmax.reshape(cfg.ncores, T, ct)
    lo = wmin3.min(axis=0)   # [T, ct]
    hi = wmax3.max(axis=0)
    mask = hi >= 0
    span = np.where(mask, hi - np.minimum(lo, P - 1) + 1, 1)
    W = int(span[:, 1:].max()) if ct > 1 else 1
    W = min(max(W, 8), P)
    woff = np.clip(np.where(mask, lo, 0), 0, P - W)   # [T, ct]
    woff[:, 0] = 0
    # coverage check: every edge's dlocal inside its chunk window
    full = (np.arange(ct) == 0)[None, :]
    wid = np.where(full, P, W)
    ok = (wmin3 >= woff[None]) & (wmax3 < woff[None] + wid[None])
    assert ok[wmax3 >= 0].all(), "window coverage failed"

    # scatter matrices, windowed: [core, tile, lane, CW]
    CW = P + (ct - 1) * W
    colbase = np.concatenate(([0], P + np.arange(ct - 1) * W))
    mtw = np.zeros((cfg.ncores, T, P, CW), dtype=np.float32)
    mt1 = np.zeros((cfg.ncores, T, P, CW), dtype=np.float32)
    e_col = colbase[e_chunk] + (e_dlocal - woff[e_tile, e_chunk])
    assert (e_col >= 0).all() and (e_col < CW).all()
    np.add.at(mtw, (e_core, e_tile, e_lane, e_col), e_w)
    np.add.at(mt1, (e_core, e_tile, e_lane, e_col), 1.0)
    mtw = mtw.astype(bf16)
    mt1 = mt1.astype(bf16)

    # gather-call index layout: group-major ([all tiles' A blocks | B blocks])
    # so pair-merged calls read contiguous columns; value at (partition p,
    # col s) = idx_linear[s*16 + p%16], replicated x8.
    idx_sb = np.zeros((cfg.ncores, P, T * ct * 8), dtype=np.int16)
    for g, ctg, off, base in ((0, ct_a, 0, 0), (1, ct_b, ct_a, T * ct_a * 8)):
        if ctg == 0:
            continue
        blk = idx[:, :, off:off + ctg, :].reshape(cfg.ncores, T, ctg * P)
        cols = blk.reshape(cfg.ncores, T, ctg * 8, 16)
        for tcol in range(ctg * 8):
            dst_col = base + np.arange(T) * (ctg * 8) + tcol
            idx_sb[:, :16, dst_col] = cols[:, :, tcol, :].transpose(0, 2, 1)
    idx_sb[:, 16:, :] = np.tile(idx_sb[:, :16, :], (1, 7, 1))

    # x table in A/B row order, bf16; pad rows -> 0 (same for all cores)
    sgrp_all, srow_all = _slot_to_table_row(np.arange(cfg.total_slots), cfg)
    trow_of_slot = np.where(sgrp_all == 0, srow_all, cfg.rows_a + srow_all)
    xfull = np.zeros((cfg.total_slots, cfg.cin), dtype=np.float32)
    xfull[trow_of_slot[slot_of]] = np.asarray(x, dtype=np.float32)
    xfull = np.ascontiguousarray(xfull.astype(bf16))

    node_of_slot = np.full(cfg.total_slots, -1, dtype=np.int64)
    node_of_slot[slot_of] = np.arange(n)

    # weights / epilogue params
    eps = 1e-5
    s1 = (np.asarray(g1) / np.sqrt(np.asarray(v1) + eps)).astype(np.float32)
    t1 = (np.asarray(beta1) + (np.asarray(b1) - np.asarray(m1)) * s1).astype(np.float32)
    s2 = (np.asarray(g2) / np.sqrt(np.asarray(v2) + eps)).astype(np.float32)
    t2 = (np.asarray(beta2) + (np.asarray(b2) - np.asarray(m2)) * s2).astype(np.float32)
    b3f = np.asarray(b3, np.float32)

    def halves(v):
        # [256] -> [P, 2] with [:, h] = v[h*128:(h+1)*128]
        return np.ascontiguousarray(np.asarray(v, np.float32).reshape(2, P).T)

    # w1t[h] = W1[h*128:(h+1)*128, :].T   [128 xf, 128 of]
    w1t = np.stack([np.asarray(W1, np.float32)[h * P:(h + 1) * P, :].T
                    for h in range(2)]).astype(bf16)
    # w2t[k] = W2.T[k*128:(k+1)*128, :]   [128 f_in, 256 out]
    w2t = np.asarray(W2, np.float32).T.reshape(2, P, HIDDEN).astype(bf16)
    w3t = np.asarray(W3, np.float32).T.reshape(2, P, HIDDEN).astype(bf16)

    in_maps = []
    for c in range(cfg.ncores):
        in_maps.append({
            "xfull": xfull,
            "idx": np.ascontiguousarray(idx_sb[c]),
            "mtw": np.ascontiguousarray(mtw[c]),
            "mt1": np.ascontiguousarray(mt1[c]),
            "w1t": w1t,
            "w2t": w2t,
            "w3t": w3t,
            "s1": halves(s1), "t1": halves(t1),
            "s2": halves(s2), "t2": halves(t2),
            "b3": halves(b3f),
        })
    return in_maps, node_of_slot, ct_a, ct_b, W, CW, woff


# ---------------------------------------------------------------------------
# Bass program
# ---------------------------------------------------------------------------

def _build(cfg: Cfg, ct_a: int, ct_b: int, W: int, CW: int, woff: np.ndarray):
    import concourse.mybir as mybir
    import concourse.tile as tile
    from concourse import bacc

    ct = ct_a + ct_b
    T, TA = cfg.tiles, cfg.tiles_a
    TB = cfg.tiles_b
    SPC = cfg.slots_per_core
    RA, RB = cfg.rows_a, cfg.rows_b
    DT = mybir.dt
    AF = mybir.ActivationFunctionType
    nc = bacc.Bacc("TRN2", target_bir_lowering=False, debug=False,
                   num_devices=cfg.ncores, num_swdge_queues=4,
                   dynamic_dma_scratch_size=24576)

    xfull_d = nc.declare_dram_parameter("xfull", [cfg.total_slots, cfg.cin], DT.bfloat16, isOutput=False)
    idx_d = nc.declare_dram_parameter("idx", [P, T * ct * 8], DT.int16, isOutput=False)
    mtw_d = nc.declare_dram_parameter("mtw", [T, P, CW], DT.bfloat16, isOutput=False)
    mt1_d = nc.declare_dram_parameter("mt1", [T, P, CW], DT.bfloat16, isOutput=False)
    w1t_d = nc.declare_dram_parameter("w1t", [2, P, P], DT.bfloat16, isOutput=False)
    w2t_d = nc.declare_dram_parameter("w2t", [2, P, HIDDEN], DT.bfloat16, isOutput=False)
    w3t_d = nc.declare_dram_parameter("w3t", [2, P, HIDDEN], DT.bfloat16, isOutput=False)
    bn_d = {}
    for nm in ("s1", "t1", "s2", "t2", "b3"):
        bn_d[nm] = nc.declare_dram_parameter(nm, [P, 2], DT.float32, isOutput=False)
    out_d = nc.declare_dram_parameter("out", [HIDDEN, SPC], DT.float32, isOutput=True)

    # z slices (this core's chunk-A / chunk-B rows) and gathered full tables
    zs = {}
    zf = {}
    for l in (2, 3):
        zs[(l, 0)] = nc.dram_tensor(f"zs{l}a", [TA * P, HIDDEN], DT.bfloat16)
        zs[(l, 1)] = nc.dram_tensor(f"zs{l}b", [TB * P, HIDDEN], DT.bfloat16)
        zf[(l, 0)] = nc.dram_tensor(f"zf{l}a", [RA, HIDDEN], DT.bfloat16,
                                    addr_space="Shared")
        zf[(l, 1)] = nc.dram_tensor(f"zf{l}b", [RB, HIDDEN], DT.bfloat16,
                                    addr_space="Shared")
    groups = [list(range(cfg.ncores))]

    def ag(l, g):
        nc.gpsimd.collective_compute(
            "AllGather", mybir.AluOpType.bypass, replica_groups=groups,
            ins=[zs[(l, g)][:]], outs=[zf[(l, g)][:]])

    with tile.TileContext(nc) as tc:
        with (
            tc.tile_pool(name="const", bufs=1) as const_pool,
            tc.tile_pool(name="mpool", bufs=6) as m_pool,
            tc.tile_pool(name="gpool", bufs=8) as g_pool,
            tc.tile_pool(name="hpool", bufs=3) as h_pool,
            tc.tile_pool(name="zpool", bufs=3) as z_pool,
            tc.tile_pool(name="opool", bufs=3) as o_pool,
            tc.tile_pool(name="apool", bufs=2) as a_pool,
            tc.tile_pool(name="agg_ps", bufs=3, space="PSUM") as agg_psum,
            tc.tile_pool(name="h_ps", bufs=2, space="PSUM") as h_psum,
            tc.tile_pool(name="z_ps", bufs=2, space="PSUM") as z_psum,
        ):
            # persistent tiles
            idx_sb = const_pool.tile([P, T * ct * 8], DT.int16)
            nc.sync.dma_start(idx_sb[:], idx_d[:])
            w1t_sb = const_pool.tile([P, 2, P], DT.bfloat16)
            nc.sync.dma_start(w1t_sb[:], w1t_d[:].rearrange("h p f -> p h f"))
            w2t_sb = const_pool.tile([P, 2, HIDDEN], DT.bfloat16)
            nc.sync.dma_start(w2t_sb[:], w2t_d[:].rearrange("h p n -> p h n"))
            w3t_sb = const_pool.tile([P, 2, HIDDEN], DT.bfloat16)
            nc.sync.dma_start(w3t_sb[:], w3t_d[:].rearrange("h p n -> p h n"))
            bn_sb = {}
            for nm in ("s1", "t1", "s2", "t2", "b3"):
                t_ = const_pool.tile([P, 2], DT.float32, tag=f"bn_{nm}")
                nc.sync.dma_start(t_[:], bn_d[nm][:])
                bn_sb[nm] = t_

            def agg_chain(t, ps, ga, ao, gb, bo, mt_sb, nfeat_half):
                """Accumulate aggT into ps: psH = ps[:, H*P:(H+1)*P]."""
                nh = nfeat_half
                for k in range(ct):
                    g = (ga[:, ao + k, :] if k < ct_a
                         else gb[:, bo + k - ct_a, :])
                    if k == 0:
                        co, wo, wd = 0, 0, P
                    else:
                        co = P + (k - 1) * W
                        wo, wd = int(woff[t, k]), W
                    for h in range(nh):
                        nc.tensor.matmul(
                            ps[:, h * P + wo: h * P + wo + wd],
                            g[:, h * P:(h + 1) * P] if nh > 1 else g,
                            mt_sb[:, co:co + wd],
                            start=(k == 0 and h == 0),
                            stop=(k == ct - 1 and h == nh - 1),
                            skip_group_check=True)

            def gather_grp(t0, nt, l, grp, elem, tag, bufs, q):
                """One gather covering `nt` consecutive tiles' group chunks."""
                if l == 1:
                    src = xfull_d[0:RA, :] if grp == 0 else xfull_d[RA:, :]
                else:
                    src = zf[(l, grp)][:]
                ctg = ct_a if grp == 0 else ct_b
                base = 0 if grp == 0 else T * ct_a * 8
                c0 = base + t0 * ctg * 8
                g = g_pool.tile([P, nt * ctg, elem], DT.bfloat16,
                                tag=f"g{'ab'[grp]}{tag}{nt}", bufs=bufs)
                nc.gpsimd.dma_gather(
                    g[:], src, idx_sb[:, c0: c0 + nt * ctg * 8],
                    nt * ctg * P, nt * ctg * P, elem, single_packet=False,
                    queue_num=q)
                return g

            def znext(t, hT, wnext_sb, zl):
                """z_{l+1} tile = h @ W.T -> DRAM slice (chunk A or B)."""
                zps = z_psum.tile([P, 512], DT.float32, tag="zps")
                for h in range(2):
                    nc.tensor.matmul(zps[:, 0:HIDDEN], hT[:, h, :],
                                     wnext_sb[:, h, :], start=(h == 0),
                                     stop=(h == 1), skip_group_check=True)
                zn = z_pool.tile([P, HIDDEN], DT.bfloat16, tag="zn")
                # copy + write both on the Activation engine so its queue is
                # self-contained and never blocks the sync queue's mt loads
                nc.scalar.copy(zn[:], zps[:, 0:HIDDEN])
                if t < TA:
                    nc.scalar.dma_start(zs[(zl, 0)][t * P:(t + 1) * P, :], zn[:])
                else:
                    nc.scalar.dma_start(zs[(zl, 1)][(t - TA) * P:(t - TA + 1) * P, :], zn[:])

            def l1_tile(t, ga, ao, gb, bo):
                mt_sb = m_pool.tile([P, CW], DT.bfloat16, tag="mt", bufs=4)
                nc.sync.dma_start(mt_sb[:], mtw_d[t])
                ps = agg_psum.tile([P, 512], DT.float32, tag="agg")
                agg_chain(t, ps, ga, ao, gb, bo, mt_sb, nfeat_half=1)
                aT = a_pool.tile([P, P], DT.bfloat16, tag="aT")
                nc.scalar.copy(aT[:], ps[:, 0:P])
                hps = h_psum.tile([P, 512], DT.float32, tag="hps")
                for h in range(2):
                    nc.tensor.matmul(hps[:, h * P:(h + 1) * P], w1t_sb[:, h, :],
                                     aT[:], start=(h == 0), stop=(h == 1),
                                     skip_group_check=True)
                hT = h_pool.tile([P, 2, P], DT.bfloat16, tag="hT")
                for h in range(2):
                    nc.scalar.activation(
                        hT[:, h, :], hps[:, h * P:(h + 1) * P], AF.Relu,
                        bias=bn_sb["t1"][:, h:h + 1], scale=bn_sb["s1"][:, h:h + 1])
                znext(t, hT, w2t_sb, 2)
                if t == TA + 12:
                    ag(2, 0)

            def l2_tile(t, ga, ao, gb, bo):
                mt_sb = m_pool.tile([P, CW], DT.bfloat16, tag="mt", bufs=4)
                nc.sync.dma_start(mt_sb[:], mtw_d[t])
                ps = agg_psum.tile([P, 512], DT.float32, tag="agg")
                agg_chain(t, ps, ga, ao, gb, bo, mt_sb, nfeat_half=2)
                hT = h_pool.tile([P, 2, P], DT.bfloat16, tag="hT")
                for h in range(2):
                    nc.scalar.activation(
                        hT[:, h, :], ps[:, h * P:(h + 1) * P], AF.Relu,
                        bias=bn_sb["t2"][:, h:h + 1], scale=bn_sb["s2"][:, h:h + 1])
                znext(t, hT, w3t_sb, 3)
                if t == TA + 12:
                    ag(3, 0)

            def l3_tile(t, ga, ao, gb, bo):
                mt_sb = m_pool.tile([P, CW], DT.bfloat16, tag="mt", bufs=4)
                nc.sync.dma_start(mt_sb[:], mt1_d[t])
                ps = agg_psum.tile([P, 512], DT.float32, tag="agg")
                agg_chain(t, ps, ga, ao, gb, bo, mt_sb, nfeat_half=2)
                ot = o_pool.tile([P, 2, P], DT.float32, tag="ot")
                for h in range(2):
                    nc.scalar.activation(
                        ot[:, h, :], ps[:, h * P:(h + 1) * P], AF.Identity,
                        bias=bn_sb["b3"][:, h:h + 1])
                for h in range(2):
                    nc.scalar.dma_start(
                        out_d[h * P:(h + 1) * P, t * P:(t + 1) * P], ot[:, h, :])

            # ---------------- layer 1 (aggregate x, then W1) ----------------
            for t in range(T):
                ga = gather_grp(t, 1, 1, 0, cfg.cin, "1", 5, (2 * t) % 4)
                gb = gather_grp(t, 1, 1, 1, cfg.cin, "1", 5, (2 * t + 1) % 4)
                l1_tile(t, ga, 0, gb, 0)
            ag(2, 1)

            # ---------------- layers 2, 3 ----------------
            for l, tile_fn in ((2, l2_tile), (3, l3_tile)):
                # pre-issue group-A gathers: they only need zf_a (AG-a, done
                # mid-previous-layer), so descgen overlaps the AG-b flight
                pend = {}
                for t in range(6):
                    pend[t] = gather_grp(t, 1, l, 0, HIDDEN, "23",
                                         8, (2 * t) % 4)
                for t in range(T):
                    if t in pend:
                        ga = pend.pop(t)
                    else:
                        ga = gather_grp(t, 1, l, 0, HIDDEN, "23",
                                        8, (2 * t) % 4)
                    gb = gather_grp(t, 1, l, 1, HIDDEN, "23",
                                    5, (2 * t + 1) % 4)
                    tile_fn(t, ga, 0, gb, 0)
                if l == 2:
                    ag(3, 1)
    nc.compile()
    return nc


# ---------------------------------------------------------------------------
# Entry point
# ---------------------------------------------------------------------------

LAST_RESULTS = None  # BassKernelResults of the most recent _run (for profiling)


def _run(cfg: Cfg, inputs: dict, trace: bool = False,
         trace_cores=None) -> np.ndarray:
    global LAST_RESULTS
    from concourse.bass_utils import run_bass_kernel_spmd

    in_maps, node_of_slot, ct_a, ct_b, W, CW, woff = _prep(cfg, **inputs)
    nc = _build(cfg, ct_a, ct_b, W, CW, woff)
    kr = run_bass_kernel_spmd(nc, in_maps, list(range(cfg.ncores)), trace=trace,
                              trace_cores=trace_cores)
    LAST_RESULTS = kr
    res = kr.results
    # out per core: [HIDDEN, SPC] feature-major; assemble and transpose
    full = np.concatenate([res[c]["out"] for c in range(cfg.ncores)], axis=1)
    out = np.empty((cfg.n, HIDDEN), dtype=np.float32)
    valid = node_of_slot >= 0
    out[node_of_slot[valid]] = full[:, valid].T
    return out


def kernel(**inputs) -> np.ndarray:
    return _run(CFG, inputs)


# revision 30
# speedup vs baseline: 1.0745x; 1.0204x over previous
"""GCN encoder (3-layer, PyG GCNConv normalize=False + BN eval + ReLU) on 8 trn2 cores.

Strategy (node/dst-sharded, graph-parallel, flipped [feat, node] aggregation):
  - Nodes remapped into 8 cores x 49 tiles x 128 slots, balanced by in-degree.
    Table rows split into group A (tiles 0-23 of each core) and group B
    (tiles 24-48) so gather indices fit int16 and AllGathers can be chunked.
  - Layer 1 aggregates x directly (agg(x@W1.T) == agg(x)@W1.T): the full
    x table is staged per core, so no z1 compute phase and no first AllGather.
  - Aggregation per dst tile: edges sorted by local dst; each 128-edge chunk's
    scatter matrix is stored as a narrow [128, W] window (dst range of the
    sorted chunk) instead of a dense [128, 128] one-hot block -- ~6x less
    scatter-matrix HBM traffic. Chunk 0 is full-width with start=True (zeroes
    the PSUM bank); later chunks accumulate into column windows.
  - Flipped layout: matmul(lhsT=gathered z chunk [128e, 128f], rhs=mt window)
    accumulates aggT [128 feat, 128 nodes] per feature half. BN scale/bias are
    then per-partition, so the whole BN+ReLU epilogue is one scalar-engine
    activation per half, and z_next = h @ W.T needs no PE transposes.
  - AllGathers (z2, z3) are split into A/B chunks; chunk A is issued mid-layer
    to overlap with the remaining tiles' compute.
"""

import math
from dataclasses import dataclass

import ml_dtypes
import numpy as np

P = 128
HIDDEN = 256


@dataclass
class Cfg:
    n: int = 50000
    e: int = 1600000
    ncores: int = 8
    tiles: int = 49   # dst node tiles of 128 slots per core
    tiles_a: int = 24  # tiles in table group A (rest in group B)
    cin: int = 128

    @property
    def tiles_b(self) -> int:
        return self.tiles - self.tiles_a

    @property
    def slots_per_core(self) -> int:
        return self.tiles * P

    @property
    def total_slots(self) -> int:
        return self.ncores * self.slots_per_core

    @property
    def rows_a(self) -> int:
        return self.ncores * self.tiles_a * P

    @property
    def rows_b(self) -> int:
        return self.ncores * self.tiles_b * P


CFG = Cfg()


# ---------------------------------------------------------------------------
# Host-side preprocessing
# ---------------------------------------------------------------------------

def _balance_nodes(indeg: np.ndarray, cfg: Cfg) -> np.ndarray:
    """Assign each node a slot in [0, total_slots) so that each 128-slot tile
    has roughly equal total in-degree. Returns slot_of_node [n]."""
    import heapq

    nbins = cfg.ncores * cfg.tiles
    order = np.argsort(-indeg, kind="stable")
    heap = [(0, b) for b in range(nbins)]
    heapq.heapify(heap)
    counts = np.zeros(nbins, dtype=np.int64)
    slot_of = np.empty(cfg.n, dtype=np.int64)
    for v in order:
        load, b = heapq.heappop(heap)
        slot_of[v] = b * P + counts[b]
        counts[b] += 1
        load += int(indeg[v])
        if counts[b] < P:
            heapq.heappush(heap, (load, b))
    return slot_of


def _slot_to_table_row(s: np.ndarray, cfg: Cfg):
    """slot -> (group 0/1, row within that group's table)"""
    c = s // cfg.slots_per_core
    r = s % cfg.slots_per_core
    t = r // P
    lane = r % P
    grp = (t >= cfg.tiles_a).astype(np.int64)
    row_a = c * (cfg.tiles_a * P) + t * P + lane
    row_b = c * (cfg.tiles_b * P) + (t - cfg.tiles_a) * P + lane
    return grp, np.where(grp == 0, row_a, row_b)


def _prep(cfg: Cfg, x, edge_index, edge_attr, W1, b1, g1, beta1, m1, v1,
          W2, b2, g2, beta2, m2, v2, W3, b3):
    bf16 = ml_dtypes.bfloat16
    n, e = cfg.n, cfg.e
    T, TA = cfg.tiles, cfg.tiles_a
    src = np.asarray(edge_index[0], dtype=np.int64)
    dst = np.asarray(edge_index[1], dtype=np.int64)
    ew = np.asarray(edge_attr, dtype=np.float32).mean(axis=1)

    indeg = np.bincount(dst, minlength=n)
    slot_of = _balance_nodes(indeg, cfg)

    # Re-label which 24 tiles of each core form table-group A so that the
    # per-dst-tile A/B edge counts are balanced (reduces max chunk counts).
    # The swap only permutes tile indices within a core: kernel structure is
    # identical on all cores (SPMD); the mapping lives in host-side data.
    sbin0 = slot_of[src] // P
    dbin0 = slot_of[dst] // P
    nb = cfg.ncores * T
    cnt = np.zeros((nb, nb), dtype=np.int32)
    np.add.at(cnt, (sbin0, dbin0), 1)
    rng = np.random.default_rng(0)
    asel = np.zeros((cfg.ncores, T), dtype=bool)
    asel[:, :TA] = True
    a_cnt = cnt[asel.reshape(-1)].sum(axis=0).astype(np.int64)
    tot_cnt = cnt.sum(axis=0).astype(np.int64)

    def score(ac):
        bc = tot_cnt - ac
        return (int(np.ceil(ac.max() / P) + np.ceil(bc.max() / P)),
                int(ac.max() + bc.max()))

    best = score(a_cnt)
    for _ in range(4000):
        c = rng.integers(cfg.ncores)
        rows = np.flatnonzero(asel[c])
        rows_b = np.flatnonzero(~asel[c])
        i = rows[rng.integers(len(rows))]
        j = rows_b[rng.integers(len(rows_b))]
        gi, gj = c * T + i, c * T + j
        new_a = a_cnt - cnt[gi] + cnt[gj]
        s = score(new_a)
        if s <= best:
            best = s
            a_cnt = new_a
            asel[c, i] = False
            asel[c, j] = True
    # permute tile indices: A-set tiles -> 0..TA-1, rest -> TA..T-1
    perm = np.empty((cfg.ncores, T), dtype=np.int64)
    for c in range(cfg.ncores):
        a_tiles = np.flatnonzero(asel[c])
        b_tiles = np.flatnonzero(~asel[c])
        perm[c, a_tiles] = np.arange(TA)
        perm[c, b_tiles] = TA + np.arange(T - TA)
    s_core = slot_of // cfg.slots_per_core
    s_tile = (slot_of % cfg.slots_per_core) // P
    s_lane = slot_of % P
    slot_of = (s_core * cfg.slots_per_core
               + perm[s_core, s_tile] * P + s_lane)

    sslot = slot_of[src]
    dslot = slot_of[dst]
    sgrp, srow = _slot_to_table_row(sslot, cfg)
    ebin = dslot // P            # global tile id
    dlocal = dslot % P

    nbins = cfg.ncores * T
    key = ebin * 2 + sgrp
    order = np.lexsort((srow, dlocal, key))
    key_s = key[order]
    counts_g = np.bincount(key_s, minlength=nbins * 2)
    gstart = np.zeros(nbins * 2, dtype=np.int64)
    gstart[1:] = np.cumsum(counts_g)[:-1]
    rank = np.arange(e, dtype=np.int64) - gstart[key_s]

    ct_a = int(math.ceil(counts_g[0::2].max() / P))
    ct_b = int(math.ceil(counts_g[1::2].max() / P))
    ct = ct_a + ct_b

    e_bin = key_s // 2
    e_grp = key_s % 2
    e_chunk = rank // P + e_grp * ct_a
    e_srow = srow[order]
    e_dlocal = dlocal[order]
    e_w = ew[order].astype(np.float32)

    # re-sort lanes within each 128-edge chunk by src row (ascending gather
    # addresses per descriptor burst); chunk membership/windows unchanged
    cid = e_bin * ct + e_chunk
    order2 = np.lexsort((e_srow, cid))
    cid = cid[order2]
    e_bin = e_bin[order2]
    e_chunk = e_chunk[order2]
    e_srow = e_srow[order2]
    e_dlocal = e_dlocal[order2]
    e_w = e_w[order2]
    cstart = np.zeros(cid.max() + 2, dtype=np.int64)
    ccnt = np.bincount(cid, minlength=cid.max() + 1)
    cstart[1:] = np.cumsum(ccnt)
    e_lane = np.arange(e, dtype=np.int64) - cstart[cid]
    e_core = e_bin // T
    e_tile = e_bin % T

    # IDX[core, tile, chunk, lane] int16, pad = 0 (valid row, weight 0)
    idx = np.zeros((cfg.ncores, T, ct, P), dtype=np.int16)
    idx[e_core, e_tile, e_chunk, e_lane] = e_srow.astype(np.int16)

    # dst-window per (core, tile, chunk); offsets must be uniform across
    # cores (SPMD single program), so take min/max over cores.
    ncid = cfg.ncores * T * ct
    cid = e_bin * ct + e_chunk
    wmin = np.full(ncid, P, dtype=np.int64)
    wmax = np.full(ncid, -1, dtype=np.int64)
    np.minimum.at(wmin, cid, e_dlocal)
    np.maximum.at(wmax, cid, e_dlocal)
    wmin3 = wmin.reshape(cfg.ncores, T, ct)
    wmax3 = wmax.reshape(cfg.ncores, T, ct)
    lo = wmin3.min(axis=0)   # [T, ct]
    hi = wmax3.max(axis=0)
    mask = hi >= 0
    span = np.where(mask, hi - np.minimum(lo, P - 1) + 1, 1)
    W = int(span[:, 1:].max()) if ct > 1 else 1
    W = min(max(W, 8), P)
    woff = np.clip(np.where(mask, lo, 0), 0, P - W)   # [T, ct]
    woff[:, 0] = 0
    # coverage check: every edge's dlocal inside its chunk window
    full = (np.arange(ct) == 0)[None, :]
    wid = np.where(full, P, W)
    ok = (wmin3 >= woff[None]) & (wmax3 < woff[None] + wid[None])
    assert ok[wmax3 >= 0].all(), "window coverage failed"

    # scatter matrices, windowed: [core, tile, lane, CW]
    CW = P + (ct - 1) * W
    colbase = np.concatenate(([0], P + np.arange(ct - 1) * W))
    mtw = np.zeros((cfg.ncores, T, P, CW), dtype=np.float32)
    mt1 = np.zeros((cfg.ncores, T, P, CW), dtype=np.float32)
    e_col = colbase[e_chunk] + (e_dlocal - woff[e_tile, e_chunk])
    assert (e_col >= 0).all() and (e_col < CW).all()
    np.add.at(mtw, (e_core, e_tile, e_lane, e_col), e_w)
    np.add.at(mt1, (e_core, e_tile, e_lane, e_col), 1.0)
    mtw = mtw.astype(bf16)
    mt1 = mt1.astype(bf16)

    # gather-call index layout: group-major ([all tiles' A blocks | B blocks])
    # so pair-merged calls read contiguous columns; value at (partition p,
    # col s) = idx_linear[s*16 + p%16], replicated x8.
    idx_sb = np.zeros((cfg.ncores, P, T * ct * 8), dtype=np.int16)
    for g, ctg, off, base in ((0, ct_a, 0, 0), (1, ct_b, ct_a, T * ct_a * 8)):
        if ctg == 0:
            continue
        blk = idx[:, :, off:off + ctg, :].reshape(cfg.ncores, T, ctg * P)
        cols = blk.reshape(cfg.ncores, T, ctg * 8, 16)
        for tcol in range(ctg * 8):
            dst_col = base + np.arange(T) * (ctg * 8) + tcol
            idx_sb[:, :16, dst_col] = cols[:, :, tcol, :].transpose(0, 2, 1)
    idx_sb[:, 16:, :] = np.tile(idx_sb[:, :16, :], (1, 7, 1))

    # x table in A/B row order, bf16; pad rows -> 0 (same for all cores)
    sgrp_all, srow_all = _slot_to_table_row(np.arange(cfg.total_slots), cfg)
    trow_of_slot = np.where(sgrp_all == 0, srow_all, cfg.rows_a + srow_all)
    xfull = np.zeros((cfg.total_slots, cfg.cin), dtype=np.float32)
    xfull[trow_of_slot[slot_of]] = np.asarray(x, dtype=np.float32)
    xfull = np.ascontiguousarray(xfull.astype(bf16))

    node_of_slot = np.full(cfg.total_slots, -1, dtype=np.int64)
    node_of_slot[slot_of] = np.arange(n)

    # weights / epilogue params
    eps = 1e-5
    s1 = (np.asarray(g1) / np.sqrt(np.asarray(v1) + eps)).astype(np.float32)
    t1 = (np.asarray(beta1) + (np.asarray(b1) - np.asarray(m1)) * s1).astype(np.float32)
    s2 = (np.asarray(g2) / np.sqrt(np.asarray(v2) + eps)).astype(np.float32)
    t2 = (np.asarray(beta2) + (np.asarray(b2) - np.asarray(m2)) * s2).astype(np.float32)
    b3f = np.asarray(b3, np.float32)

    def halves(v):
        # [256] -> [P, 2] with [:, h] = v[h*128:(h+1)*128]
        return np.ascontiguousarray(np.asarray(v, np.float32).reshape(2, P).T)

    # w1t[h] = W1[h*128:(h+1)*128, :].T   [128 xf, 128 of]
    w1t = np.stack([np.asarray(W1, np.float32)[h * P:(h + 1) * P, :].T
                    for h in range(2)]).astype(bf16)
    # w2t[k] = W2.T[k*128:(k+1)*128, :]   [128 f_in, 256 out]
    w2t = np.asarray(W2, np.float32).T.reshape(2, P, HIDDEN).astype(bf16)
    w3t = np.asarray(W3, np.float32).T.reshape(2, P, HIDDEN).astype(bf16)

    in_maps = []
    for c in range(cfg.ncores):
        in_maps.append({
            "xfull": xfull,
            "idx": np.ascontiguousarray(idx_sb[c]),
            "mtw": np.ascontiguousarray(mtw[c]),
            "mt1": np.ascontiguousarray(mt1[c]),
            "w1t": w1t,
            "w2t": w2t,
            "w3t": w3t,
            "s1": halves(s1), "t1": halves(t1),
            "s2": halves(s2), "t2": halves(t2),
            "b3": halves(b3f),
        })
    return in_maps, node_of_slot, ct_a, ct_b, W, CW, woff


# ---------------------------------------------------------------------------
# Bass program
# ---------------------------------------------------------------------------

def _build(cfg: Cfg, ct_a: int, ct_b: int, W: int, CW: int, woff: np.ndarray):
    import concourse.mybir as mybir
    import concourse.tile as tile
    from concourse import bacc

    ct = ct_a + ct_b
    T, TA = cfg.tiles, cfg.tiles_a
    TB = cfg.tiles_b
    SPC = cfg.slots_per_core
    RA, RB = cfg.rows_a, cfg.rows_b
    DT = mybir.dt
    AF = mybir.ActivationFunctionType
    nc = bacc.Bacc("TRN2", target_bir_lowering=False, debug=False,
                   num_devices=cfg.ncores, num_swdge_queues=4)

    xfull_d = nc.declare_dram_parameter("xfull", [cfg.total_slots, cfg.cin], DT.bfloat16, isOutput=False)
    idx_d = nc.declare_dram_parameter("idx", [P, T * ct * 8], DT.int16, isOutput=False)
    mtw_d = nc.declare_dram_parameter("mtw", [T, P, CW], DT.bfloat16, isOutput=False)
    mt1_d = nc.declare_dram_parameter("mt1", [T, P, CW], DT.bfloat16, isOutput=False)
    w1t_d = nc.declare_dram_parameter("w1t", [2, P, P], DT.bfloat16, isOutput=False)
    w2t_d = nc.declare_dram_parameter("w2t", [2, P, HIDDEN], DT.bfloat16, isOutput=False)
    w3t_d = nc.declare_dram_parameter("w3t", [2, P, HIDDEN], DT.bfloat16, isOutput=False)
    bn_d = {}
    for nm in ("s1", "t1", "s2", "t2", "b3"):
        bn_d[nm] = nc.declare_dram_parameter(nm, [P, 2], DT.float32, isOutput=False)
    out_d = nc.declare_dram_parameter("out", [HIDDEN, SPC], DT.float32, isOutput=True)

    # z slices (this core's chunk-A / chunk-B rows) and gathered full tables
    zs = {}
    zf = {}
    for l in (2, 3):
        zs[(l, 0)] = nc.dram_tensor(f"zs{l}a", [TA * P, HIDDEN], DT.bfloat16)
        zs[(l, 1)] = nc.dram_tensor(f"zs{l}b", [TB * P, HIDDEN], DT.bfloat16)
        zf[(l, 0)] = nc.dram_tensor(f"zf{l}a", [RA, HIDDEN], DT.bfloat16,
                                    addr_space="Shared")
        zf[(l, 1)] = nc.dram_tensor(f"zf{l}b", [RB, HIDDEN], DT.bfloat16,
                                    addr_space="Shared")
    groups = [list(range(cfg.ncores))]

    def ag(l, g):
        nc.gpsimd.collective_compute(
            "AllGather", mybir.AluOpType.bypass, replica_groups=groups,
            ins=[zs[(l, g)][:]], outs=[zf[(l, g)][:]])

    with tile.TileContext(nc) as tc:
        with (
            tc.tile_pool(name="const", bufs=1) as const_pool,
            tc.tile_pool(name="mpool", bufs=6) as m_pool,
            tc.tile_pool(name="gpool", bufs=8) as g_pool,
            tc.tile_pool(name="hpool", bufs=3) as h_pool,
            tc.tile_pool(name="zpool", bufs=3) as z_pool,
            tc.tile_pool(name="opool", bufs=3) as o_pool,
            tc.tile_pool(name="apool", bufs=2) as a_pool,
            tc.tile_pool(name="agg_ps", bufs=3, space="PSUM") as agg_psum,
            tc.tile_pool(name="h_ps", bufs=2, space="PSUM") as h_psum,
            tc.tile_pool(name="z_ps", bufs=2, space="PSUM") as z_psum,
        ):
            # persistent tiles
            idx_sb = const_pool.tile([P, T * ct * 8], DT.int16)
            nc.sync.dma_start(idx_sb[:], idx_d[:])
            w1t_sb = const_pool.tile([P, 2, P], DT.bfloat16)
            nc.sync.dma_start(w1t_sb[:], w1t_d[:].rearrange("h p f -> p h f"))
            w2t_sb = const_pool.tile([P, 2, HIDDEN], DT.bfloat16)
            nc.sync.dma_start(w2t_sb[:], w2t_d[:].rearrange("h p n -> p h n"))
            w3t_sb = const_pool.tile([P, 2, HIDDEN], DT.bfloat16)
            nc.sync.dma_start(w3t_sb[:], w3t_d[:].rearrange("h p n -> p h n"))
            bn_sb = {}
            for nm in ("s1", "t1", "s2", "t2", "b3"):
                t_ = const_pool.tile([P, 2], DT.float32, tag=f"bn_{nm}")
                nc.sync.dma_start(t_[:], bn_d[nm][:])
                bn_sb[nm] = t_

            def agg_chain(t, ps, ga, ao, gb, bo, mt_sb, nfeat_half):
                """Accumulate aggT into ps: psH = ps[:, H*P:(H+1)*P]."""
                nh = nfeat_half
                for k in range(ct):
                    g = (ga[:, ao + k, :] if k < ct_a
                         else gb[:, bo + k - ct_a, :])
                    if k == 0:
                        co, wo, wd = 0, 0, P
                    else:
                        co = P + (k - 1) * W
                        wo, wd = int(woff[t, k]), W
                    for h in range(nh):
                        nc.tensor.matmul(
                            ps[:, h * P + wo: h * P + wo + wd],
                            g[:, h * P:(h + 1) * P] if nh > 1 else g,
                            mt_sb[:, co:co + wd],
                            start=(k == 0 and h == 0),
                            stop=(k == ct - 1 and h == nh - 1),
                            skip_group_check=True)

            def gather_grp(t0, nt, l, grp, elem, tag, bufs, q):
                """One gather covering `nt` consecutive tiles' group chunks."""
                if l == 1:
                    src = xfull_d[0:RA, :] if grp == 0 else xfull_d[RA:, :]
                else:
                    src = zf[(l, grp)][:]
                ctg = ct_a if grp == 0 else ct_b
                base = 0 if grp == 0 else T * ct_a * 8
                c0 = base + t0 * ctg * 8
                g = g_pool.tile([P, nt * ctg, elem], DT.bfloat16,
                                tag=f"g{'ab'[grp]}{tag}{nt}", bufs=bufs)
                nc.gpsimd.dma_gather(
                    g[:], src, idx_sb[:, c0: c0 + nt * ctg * 8],
                    nt * ctg * P, nt * ctg * P, elem, single_packet=False,
                    queue_num=q)
                return g

            def znext(t, hT, wnext_sb, zl):
                """z_{l+1} tile = h @ W.T -> DRAM slice (chunk A or B)."""
                zps = z_psum.tile([P, 512], DT.float32, tag="zps")
                for h in range(2):
                    nc.tensor.matmul(zps[:, 0:HIDDEN], hT[:, h, :],
                                     wnext_sb[:, h, :], start=(h == 0),
                                     stop=(h == 1), skip_group_check=True)
                zn = z_pool.tile([P, HIDDEN], DT.bfloat16, tag="zn")
                # copy + write both on the Activation engine so its queue is
                # self-contained and never blocks the sync queue's mt loads
                nc.scalar.copy(zn[:], zps[:, 0:HIDDEN])
                if t < TA:
                    nc.scalar.dma_start(zs[(zl, 0)][t * P:(t + 1) * P, :], zn[:])
                else:
                    nc.scalar.dma_start(zs[(zl, 1)][(t - TA) * P:(t - TA + 1) * P, :], zn[:])

            def l1_tile(t, ga, ao, gb, bo):
                mt_sb = m_pool.tile([P, CW], DT.bfloat16, tag="mt", bufs=6)
                nc.sync.dma_start(mt_sb[:], mtw_d[t])
                ps = agg_psum.tile([P, 512], DT.float32, tag="agg")
                agg_chain(t, ps, ga, ao, gb, bo, mt_sb, nfeat_half=1)
                aT = a_pool.tile([P, P], DT.bfloat16, tag="aT")
                nc.scalar.copy(aT[:], ps[:, 0:P])
                hps = h_psum.tile([P, 512], DT.float32, tag="hps")
                for h in range(2):
                    nc.tensor.matmul(hps[:, h * P:(h + 1) * P], w1t_sb[:, h, :],
                                     aT[:], start=(h == 0), stop=(h == 1),
                                     skip_group_check=True)
                hT = h_pool.tile([P, 2, P], DT.bfloat16, tag="hT")
                for h in range(2):
                    nc.scalar.activation(
                        hT[:, h, :], hps[:, h * P:(h + 1) * P], AF.Relu,
                        bias=bn_sb["t1"][:, h:h + 1], scale=bn_sb["s1"][:, h:h + 1])
                znext(t, hT, w2t_sb, 2)
                if t == TA + 12:
                    ag(2, 0)

            def l2_tile(t, ga, ao, gb, bo):
                mt_sb = m_pool.tile([P, CW], DT.bfloat16, tag="mt", bufs=6)
                nc.sync.dma_start(mt_sb[:], mtw_d[t])
                ps = agg_psum.tile([P, 512], DT.float32, tag="agg")
                agg_chain(t, ps, ga, ao, gb, bo, mt_sb, nfeat_half=2)
                hT = h_pool.tile([P, 2, P], DT.bfloat16, tag="hT")
                for h in range(2):
                    nc.scalar.activation(
                        hT[:, h, :], ps[:, h * P:(h + 1) * P], AF.Relu,
                        bias=bn_sb["t2"][:, h:h + 1], scale=bn_sb["s2"][:, h:h + 1])
                znext(t, hT, w3t_sb, 3)
                if t == TA + 12:
                    ag(3, 0)

            def l3_tile(t, ga, ao, gb, bo):
                mt_sb = m_pool.tile([P, CW], DT.bfloat16, tag="mt", bufs=6)
                nc.sync.dma_start(mt_sb[:], mt1_d[t])
                ps = agg_psum.tile([P, 512], DT.float32, tag="agg")
                agg_chain(t, ps, ga, ao, gb, bo, mt_sb, nfeat_half=2)
                ot = o_pool.tile([P, 2, P], DT.float32, tag="ot")
                for h in range(2):
                    nc.scalar.activation(
                        ot[:, h, :], ps[:, h * P:(h + 1) * P], AF.Identity,
                        bias=bn_sb["b3"][:, h:h + 1])
                for h in range(2):
                    nc.scalar.dma_start(
                        out_d[h * P:(h + 1) * P, t * P:(t + 1) * P], ot[:, h, :])

            # ---------------- layer 1 (aggregate x, then W1) ----------------
            for t in range(T):
                ga = gather_grp(t, 1, 1, 0, cfg.cin, "1", 5, (2 * t) % 4)
                gb = gather_grp(t, 1, 1, 1, cfg.cin, "1", 5, (2 * t + 1) % 4)
                l1_tile(t, ga, 0, gb, 0)
            ag(2, 1)

            # ---------------- layers 2, 3 ----------------
            for l, tile_fn in ((2, l2_tile), (3, l3_tile)):
                # pre-issue group-A gathers: they only need zf_a (AG-a, done
                # mid-previous-layer), so descgen overlaps the AG-b flight
                pend = {}
                for t in range(6):
                    pend[t] = gather_grp(t, 1, l, 0, HIDDEN, "23",
                                         8, (2 * t) % 4)
                for t in range(T):
                    if t in pend:
                        ga = pend.pop(t)
                    else:
                        ga = gather_grp(t, 1, l, 0, HIDDEN, "23",
                                        8, (2 * t) % 4)
                    gb = gather_grp(t, 1, l, 1, HIDDEN, "23",
                                    5, (2 * t + 1) % 4)
                    tile_fn(t, ga, 0, gb, 0)
                if l == 2:
                    ag(3, 1)
    nc.compile()
    return nc


# ---------------------------------------------------------------------------
# Entry point
# ---------------------------------------------------------------------------

LAST_RESULTS = None  # BassKernelResults of the most recent _run (for profiling)


def _run(cfg: Cfg, inputs: dict, trace: bool = False,
         trace_cores=None) -> np.ndarray:
    global LAST_RESULTS
    from concourse.bass_utils import run_bass_kernel_spmd

    in_maps, node_of_slot, ct_a, ct_b, W, CW, woff = _prep(cfg, **inputs)
    nc = _build(cfg, ct_a, ct_b, W, CW, woff)
    kr = run_bass_kernel_spmd(nc, in_maps, list(range(cfg.ncores)), trace=trace,
                              trace_cores=trace_cores)
    LAST_RESULTS = kr
    res = kr.results
    # out per core: [HIDDEN, SPC] feature-major; assemble and transpose
    full = np.concatenate([res[c]["out"] for c in range(cfg.ncores)], axis=1)
    out = np.empty((cfg.n, HIDDEN), dtype=np.float32)
    valid = node_of_slot >= 0
    out[node_of_slot[valid]] = full[:, valid].T
    return out


def kernel(**inputs) -> np.ndarray:
    return _run(CFG, inputs)
